# revision 1
# baseline (speedup 1.0000x reference)
"""MipNerf IPE encoding kernel for Trainium2 (Bass/Tile), 8-core SPMD.

Computes reference(ray_o, ray_d, fg_z_vals, bg_z_vals, radii) -> [2048, 64, 768]:
  fg: diagonal-cov cone cast + diagonal IPE (48 sin + 48 cos features)
  bg: full-cov cone cast + contraction Jacobian + icosahedral-basis IPE
      (336 sin + 336 cos features)

Sharding: embarrassingly data-parallel over rays; 256 rays per core.

Key implementation notes:
  - All per-sample quantities are computed with partition = ray (128 rays /
    tile, 2 tiles per core), free dim = samples (64) x feature.
  - The contraction algebra is reduced to closed-form scalars:
      J = a*I + b*x*x^T, a = 2/n - 1/n^2, b = 2*rn^2*rn0*(rn-1)
      yv0_q = A1*e_q^2 + A2*(e_q c_q) + A3*c_q^2 + A4*w_q,  y0_q = a*c_q
  - sin/cos use the hardware `sin2pi` spline (valid on [-0.5, 0.5]) with an
    int32 fixed-point angle: u = round(frac_centered(y0/2pi) * 2^31).
    Scaling by 2^j is an exact wrapping left-shift.  cos(2pi g) is
    sin2pi(0.25 - |g|), which stays in range without a wrapping add.
  - exp uses the per-level ACT scale immediate (-0.5 * 4^j); sin2pi and exp
    share one ACT table set (exp_and_friends), so no table switches.
  - rsqrt/reciprocal: ACT Rsqrt/Reciprocal are banned (accuracy); rsqrt is a
    magic-constant seed + 4 Newton iterations on DVE, reciprocal is the DVE
    iterative divide.
"""

import numpy as np

import concourse.bass as bass
import concourse.tile as tile
from concourse import mybir
from concourse.vector_clock import ScopedClock

F32 = mybir.dt.float32
I32 = mybir.dt.int32
U32 = mybir.dt.uint32
AF = mybir.ActivationFunctionType
OP = mybir.AluOpType

MAGIC_RND = 12582912.0          # 1.5 * 2^23, float32 round-to-nearest trick
RSQRT_MAGIC = 0x5F3759DF
INV2PI = float(1.0 / (2.0 * np.pi))
TINY = 1e-6

# icosahedral basis (matches reference.py)
P_BASIS = np.array([
    0.8506508, 0.0, 0.5257311, 0.809017, 0.5, 0.309017, 0.5257311, 0.8506508, 0.0,
    1.0, 0.0, 0.0, 0.809017, 0.5, -0.309017, 0.8506508, 0.0, -0.5257311, 0.309017,
    0.809017, -0.5, 0.0, 0.5257311, -0.8506508, 0.5, 0.309017, -0.809017, 0.0, 1.0,
    0.0, -0.5257311, 0.8506508, 0.0, -0.309017, 0.809017, -0.5, 0.0, 0.5257311,
    0.8506508, -0.309017, 0.809017, 0.5, 0.309017, 0.809017, 0.5, 0.5, 0.309017,
    0.809017, 0.5, -0.309017, 0.809017, 0.0, 0.0, 1.0, -0.5, 0.309017, 0.809017,
    -0.809017, 0.5, 0.309017, -0.809017, 0.5, -0.309017], dtype=np.float32).reshape(3, 21)

N_CORES = 8
RAYS_PER_CORE = 256
NS = 64           # samples per ray
NL = 16           # frequency levels
NF = 24           # 21 bg basis dims + 3 fg axes
HALF = 32         # samples per half-tile
BLK = 8           # samples per output block
FOUT = 768


# ---------------------------------------------------------------------------
# walrus workarounds
# ---------------------------------------------------------------------------

_PATCHED = False


def _apply_patches():
    """1) split >1 sem-waits per instruction (this walrus rejects multi-wait
    instructions);  2) rewrite sentinel Arctan activations into Sin2pi."""
    global _PATCHED
    if _PATCHED:
        return
    _PATCHED = True

    import concourse.bass2jax as bass2jax

    orig_compile = bass2jax.compile_bir_kernel

    def patched_compile(bir_json, tmpdir, neff_name="file.neff"):
        if isinstance(bir_json, bytes):
            bir_json = bir_json.replace(b'"func":"Arctan"', b'"func":"Sin2pi"')
        else:
            bir_json = bir_json.replace('"func":"Arctan"', '"func":"Sin2pi"')
        return orig_compile(bir_json, tmpdir, neff_name=neff_name)

    bass2jax.compile_bir_kernel = patched_compile


_waitsplit_ctr = [0]


def _split_sync_waits(nc, max_waits=1):
    n_split = 0
    for fn in nc.m.functions:
        for bb in fn.blocks:
            il = bb.instructions
            i = 0
            while i < len(il):
                ins = il[i]
                si = ins.sync_info
                waits = list(si.on_wait) if si is not None else []
                if len(waits) > max_waits:
                    extra, keep = waits[:-max_waits], waits[-max_waits:]
                    pos = i
                    for j in range(0, len(extra), max_waits):
                        chunk = extra[j:j + max_waits]
                        _waitsplit_ctr[0] += 1
                        nop = mybir.InstNoOp(
                            name=f"waitsplit_{_waitsplit_ctr[0]}", ins=[], outs=[])
                        nop.engine = ins.engine
                        nop.sync_info = mybir.SyncInfo(on_wait=chunk, on_update=[])
                        nc.register_instruction(nop, overwrite=True)
                        il.insert(pos, nop)
                        pos += 1
                        i += 1
                    ins.sync_info = mybir.SyncInfo(
                        on_wait=keep, on_update=list(si.on_update))
                    n_split += 1
                i += 1
    return n_split


# ---------------------------------------------------------------------------
# AP helpers
# ---------------------------------------------------------------------------

def _ap(base, offset_elems, dims):
    """Custom AP over a tile/AP: keep partition dim, replace free dims."""
    return bass.AP(tensor=base.tensor, offset=base.offset + offset_elems,
                   ap=[base.ap[0]] + [list(d) for d in dims])


# ---------------------------------------------------------------------------
# kernel body
# ---------------------------------------------------------------------------

def _moments(nc, cols, wide, z, r2, out_tm2, out_tv, out_rv):
    """Frustum moments from z [128, 65] -> t_mean2 (=2*t_mean), t_var, r_var
    [128, 64].  r2 = radii^2 per-ray [128, 1]."""
    t0 = z[:, 0:NS]
    t1 = z[:, 1:NS + 1]
    sm = wide.tile([128, NS], F32, tag="mo_a")
    nc.vector.tensor_tensor(out=sm[:], in0=t0, in1=t1, op=OP.add)
    df = wide.tile([128, NS], F32, tag="mo_b")
    nc.vector.tensor_tensor(out=df[:], in0=t1, in1=t0, op=OP.subtract)
    sm2 = wide.tile([128, NS], F32, tag="mo_c")
    nc.vector.tensor_tensor(out=sm2[:], in0=sm[:], in1=sm[:], op=OP.mult)
    df2 = wide.tile([128, NS], F32, tag="mo_d")
    nc.vector.tensor_tensor(out=df2[:], in0=df[:], in1=df[:], op=OP.mult)
    # denom4 = 3*sm2 + df2
    den4 = wide.tile([128, NS], F32, tag="mo_e")
    nc.vector.scalar_tensor_tensor(out=den4[:], in0=sm2[:], scalar=3.0,
                                   in1=df2[:], op0=OP.mult, op1=OP.add)
    rden4 = wide.tile([128, NS], F32, tag="mo_f")
    nc.vector.reciprocal(out=rden4[:], in_=den4[:])
    u1 = wide.tile([128, NS], F32, tag="mo_g")
    nc.vector.tensor_tensor(out=u1[:], in0=df2[:], in1=rden4[:], op=OP.mult)
    # t_mean2 = sm * (1 + 2*u1)
    tmp = wide.tile([128, NS], F32, tag="mo_h")
    nc.vector.tensor_scalar(out=tmp[:], in0=u1[:], scalar1=2.0, scalar2=1.0,
                            op0=OP.mult, op1=OP.add)
    nc.vector.tensor_tensor(out=out_tm2[:], in0=sm[:], in1=tmp[:], op=OP.mult)
    # t_var = df2/12 - (4/15) * u1^2 * (den4 - 1.25*df2)
    u1sq = wide.tile([128, NS], F32, tag="mo_h")
    nc.vector.tensor_tensor(out=u1sq[:], in0=u1[:], in1=u1[:], op=OP.mult)
    g2 = wide.tile([128, NS], F32, tag="mo_a")
    nc.vector.scalar_tensor_tensor(out=g2[:], in0=df2[:], scalar=-1.25,
                                   in1=den4[:], op0=OP.mult, op1=OP.add)
    g3 = wide.tile([128, NS], F32, tag="mo_c")
    nc.vector.tensor_tensor(out=g3[:], in0=u1sq[:], in1=g2[:], op=OP.mult)
    g5 = wide.tile([128, NS], F32, tag="mo_e")
    nc.vector.tensor_scalar_mul(out=g5[:], in0=df2[:], scalar1=float(1.0 / 12.0))
    nc.vector.scalar_tensor_tensor(out=out_tv[:], in0=g3[:], scalar=float(-4.0 / 15.0),
                                   in1=g5[:], op0=OP.mult, op1=OP.add)
    # r_var = r2 * (sm2/16 + (5/48)*df2 - (1/15)*u1*df2)
    h1 = wide.tile([128, NS], F32, tag="mo_a")
    nc.vector.tensor_tensor(out=h1[:], in0=u1[:], in1=df2[:], op=OP.mult)
    h2 = wide.tile([128, NS], F32, tag="mo_c")
    nc.vector.tensor_scalar_mul(out=h2[:], in0=sm2[:], scalar1=float(1.0 / 16.0))
    h4 = wide.tile([128, NS], F32, tag="mo_e")
    nc.vector.scalar_tensor_tensor(out=h4[:], in0=df2[:], scalar=float(5.0 / 48.0),
                                   in1=h2[:], op0=OP.mult, op1=OP.add)
    h5 = wide.tile([128, NS], F32, tag="mo_a")
    nc.vector.scalar_tensor_tensor(out=h5[:], in0=h1[:], scalar=float(-1.0 / 15.0),
                                   in1=h4[:], op0=OP.mult, op1=OP.add)
    nc.vector.tensor_scalar_mul(out=out_rv[:], in0=h5[:], scalar1=r2[:])


def build_kernel():
    """Build the 8-core SPMD Bass module (per-core: 256 rays)."""
    _apply_patches()
    nc = bass.Bass(dynamic_dma_scratch_size=4096)

    ray_o = nc.dram_tensor("ray_o", [RAYS_PER_CORE, 3], F32, kind="ExternalInput")
    ray_d = nc.dram_tensor("ray_d", [RAYS_PER_CORE, 3], F32, kind="ExternalInput")
    fg_z = nc.dram_tensor("fg_z", [RAYS_PER_CORE, NS + 1], F32, kind="ExternalInput")
    bg_z = nc.dram_tensor("bg_z", [RAYS_PER_CORE, NS + 1], F32, kind="ExternalInput")
    radii = nc.dram_tensor("radii", [RAYS_PER_CORE, 1], F32, kind="ExternalInput")
    pconst = nc.dram_tensor("pconst", [1, 84], F32, kind="ExternalInput")
    out = nc.dram_tensor("out", [RAYS_PER_CORE, NS * FOUT], F32, kind="ExternalOutput")

    with tile.TileContext(nc) as tc:
        import contextlib
        ctx = contextlib.ExitStack()
        with ctx:
            consts = ctx.enter_context(tc.tile_pool(name="consts", bufs=1))
            cols = ctx.enter_context(tc.tile_pool(name="cols", bufs=1))
            wide = ctx.enter_context(tc.tile_pool(name="wide", bufs=2))
            base = ctx.enter_context(tc.tile_pool(name="base", bufs=1))
            big = ctx.enter_context(tc.tile_pool(name="big", bufs=1))
            ubp = ctx.enter_context(tc.tile_pool(name="ubp", bufs=2))
            yp = ctx.enter_context(tc.tile_pool(name="yp", bufs=1))
            sc = ctx.enter_context(tc.tile_pool(name="sc", bufs=2))
            ycp = ctx.enter_context(tc.tile_pool(name="ycp", bufs=1))
            outp = ctx.enter_context(tc.tile_pool(name="outp", bufs=2))

            # constants
            pc = consts.tile([128, 84], F32)
            pca = pconst[:, :]
            nc.sync.dma_start(out=pc[:], in_=bass.AP(
                tensor=pca.tensor, offset=pca.offset, ap=[[0, 128], [1, 84]]))
            magic_u = consts.tile([128, 1], U32)
            nc.vector.memset(magic_u, RSQRT_MAGIC)
            quarter = consts.tile([128, 1], F32)
            nc.vector.memset(quarter, 0.25)

            for t in range(2):
                r0 = t * 128

                # ---------------- load inputs ----------------
                zf = base.tile([128, NS + 1], F32, tag="zf")
                nc.sync.dma_start(out=zf[:], in_=fg_z[r0:r0 + 128, :])
                zb = base.tile([128, NS + 1], F32, tag="zb")
                nc.sync.dma_start(out=zb[:], in_=bg_z[r0:r0 + 128, :])
                o3 = base.tile([128, 3], F32, tag="o3")
                nc.sync.dma_start(out=o3[:], in_=ray_o[r0:r0 + 128, :])
                d3 = base.tile([128, 3], F32, tag="d3")
                nc.sync.dma_start(out=d3[:], in_=ray_d[r0:r0 + 128, :])
                rad = base.tile([128, 1], F32, tag="rad")
                nc.sync.dma_start(out=rad[:], in_=radii[r0:r0 + 128, :])

                # ---------------- per-ray scalars ----------------
                r2 = cols.tile([128, 1], F32, tag=f"r2")
                nc.vector.tensor_tensor(out=r2[:], in0=rad[:], in1=rad[:], op=OP.mult)
                dk2 = cols.tile([128, 3], F32, tag=f"dk2")
                nc.vector.tensor_tensor(out=dk2[:], in0=d3[:], in1=d3[:], op=OP.mult)
                dmag = cols.tile([128, 1], F32, tag=f"dmag")
                nc.vector.tensor_tensor(out=dmag[:], in0=dk2[:, 0:1], in1=dk2[:, 1:2], op=OP.add)
                nc.vector.tensor_tensor(out=dmag[:], in0=dmag[:], in1=dk2[:, 2:3], op=OP.add)
                nc.vector.tensor_scalar_max(out=dmag[:], in0=dmag[:], scalar1=1e-8)
                rdmag = cols.tile([128, 1], F32, tag=f"rdmag")
                nc.vector.reciprocal(out=rdmag[:], in_=dmag[:])
                hd3 = cols.tile([128, 3], F32, tag=f"hd3")
                nc.vector.tensor_scalar_mul(out=hd3[:], in0=d3[:], scalar1=0.5)

                # e = d @ P  [128, 21], esq
                e21 = cols.tile([128, 21], F32, tag=f"e21")
                nc.vector.tensor_scalar_mul(out=e21[:], in0=pc[:, 0:21], scalar1=d3[:, 0:1])
                tmp21 = cols.tile([128, 21], F32, tag="tmp21")
                nc.vector.tensor_scalar_mul(out=tmp21[:], in0=pc[:, 21:42], scalar1=d3[:, 1:2])
                nc.vector.tensor_tensor(out=e21[:], in0=e21[:], in1=tmp21[:], op=OP.add)
                nc.vector.tensor_scalar_mul(out=tmp21[:], in0=pc[:, 42:63], scalar1=d3[:, 2:3])
                nc.vector.tensor_tensor(out=e21[:], in0=e21[:], in1=tmp21[:], op=OP.add)
                esq = cols.tile([128, 21], F32, tag=f"esq")
                nc.vector.tensor_tensor(out=esq[:], in0=e21[:], in1=e21[:], op=OP.mult)

                # ---------------- moments ----------------
                tm2f = cols.tile([128, NS], F32, tag=f"tm2f")
                tvf = cols.tile([128, NS], F32, tag=f"tvf")
                rvf = cols.tile([128, NS], F32, tag=f"rvf")
                _moments(nc, cols, wide, zf, r2, tm2f, tvf, rvf)
                tm2b = cols.tile([128, NS], F32, tag=f"tm2b")
                tvb = cols.tile([128, NS], F32, tag=f"tvb")
                rvb = cols.tile([128, NS], F32, tag=f"rvb")
                _moments(nc, cols, wide, zb, r2, tm2b, tvb, rvb)

                yb = base.tile([128, NF * NS], F32, tag="ybase")    # [f*64+s]
                yvb = base.tile([128, NF * NS], F32, tag="yvbase")

                # ---------------- fg: mean + cov_diag ----------------
                alf = wide.tile([128, NS], F32, tag="mo_b")
                nc.vector.tensor_scalar_mul(out=alf[:], in0=rvf[:], scalar1=rdmag[:])
                nc.vector.tensor_tensor(out=alf[:], in0=tvf[:], in1=alf[:], op=OP.subtract)
                mk = base.tile([128, 3 * NS], F32, tag="X")
                ck = base.tile([128, 3 * NS], F32, tag="w1")
                for k in range(3):
                    # m_k = tm2f * halfd_k + o_k
                    nc.vector.tensor_scalar(
                        out=mk[:, k * NS:(k + 1) * NS], in0=tm2f[:],
                        scalar1=hd3[:, k:k + 1], scalar2=o3[:, k:k + 1],
                        op0=OP.mult, op1=OP.add)
                    # cd_k = alf * dk2_k + rvf
                    nc.vector.scalar_tensor_tensor(
                        out=ck[:, k * NS:(k + 1) * NS], in0=alf[:],
                        scalar=dk2[:, k:k + 1], in1=rvf[:], op0=OP.mult, op1=OP.add)
                # transpose k-major -> s-major cols 21..23 of yb/yvb
                nc.vector.tensor_copy(
                    out=_ap(yb[:], 21, [[NF, NS], [1, 3]]),
                    in_=_ap(mk[:], 0, [[1, NS], [NS, 3]]))
                nc.vector.tensor_copy(
                    out=_ap(yvb[:], 21, [[NF, NS], [1, 3]]),
                    in_=_ap(ck[:], 0, [[1, NS], [NS, 3]]))

                # ---------------- bg: contraction scalars ----------------
                X = base.tile([128, 3 * NS], F32, tag="X")          # [k*64+s]
                for k in range(3):
                    nc.vector.tensor_scalar(
                        out=X[:, k * NS:(k + 1) * NS], in0=tm2b[:],
                        scalar1=hd3[:, k:k + 1], scalar2=o3[:, k:k + 1],
                        op0=OP.mult, op1=OP.add)
                s2 = cols.tile([128, NS], F32, tag=f"s2")
                nc.vector.tensor_tensor(out=s2[:], in0=X[:, 0:NS], in1=X[:, 0:NS], op=OP.mult)
                w0 = wide.tile([128, NS], F32, tag="mo_a")
                nc.vector.tensor_tensor(out=w0[:], in0=X[:, NS:2 * NS], in1=X[:, NS:2 * NS], op=OP.mult)
                nc.vector.tensor_tensor(out=s2[:], in0=s2[:], in1=w0[:], op=OP.add)
                nc.vector.tensor_tensor(out=w0[:], in0=X[:, 2 * NS:3 * NS], in1=X[:, 2 * NS:3 * NS], op=OP.mult)
                nc.vector.tensor_tensor(out=s2[:], in0=s2[:], in1=w0[:], op=OP.add)
                # h = d . X
                h = cols.tile([128, NS], F32, tag=f"h")
                nc.vector.tensor_scalar_mul(out=h[:], in0=X[:, 0:NS], scalar1=d3[:, 0:1])
                nc.vector.scalar_tensor_tensor(out=h[:], in0=X[:, NS:2 * NS],
                                               scalar=d3[:, 1:2], in1=h[:],
                                               op0=OP.mult, op1=OP.add)
                nc.vector.scalar_tensor_tensor(out=h[:], in0=X[:, 2 * NS:3 * NS],
                                               scalar=d3[:, 2:3], in1=h[:],
                                               op0=OP.mult, op1=OP.add)

                # rsqrt(s2): magic seed + 4 Newton iterations
                rn0 = cols.tile([128, NS], F32, tag=f"rn0")
                seed_u = wide.tile([128, NS], U32, tag="mo_a")
                nc.vector.tensor_scalar(out=seed_u[:], in0=s2[:].bitcast(U32),
                                        scalar1=1, scalar2=None,
                                        op0=OP.logical_shift_right)
                nc.vector.tensor_tensor(
                    out=rn0[:].bitcast(U32),
                    in0=_ap(magic_u[:], 0, [[0, NS]]),
                    in1=seed_u[:], op=OP.subtract)
                for _ in range(4):
                    nr = wide.tile([128, NS], F32, tag="mo_b")
                    nc.vector.tensor_tensor(out=nr[:], in0=s2[:], in1=rn0[:], op=OP.mult)
                    nc.vector.tensor_tensor(out=nr[:], in0=nr[:], in1=rn0[:], op=OP.mult)
                    nc.vector.tensor_scalar(out=nr[:], in0=nr[:], scalar1=-0.5,
                                            scalar2=1.5, op0=OP.mult, op1=OP.add)
                    nc.vector.tensor_tensor(out=rn0[:], in0=rn0[:], in1=nr[:], op=OP.mult)

                n0 = cols.tile([128, NS], F32, tag=f"n0")
                nc.vector.tensor_tensor(out=n0[:], in0=s2[:], in1=rn0[:], op=OP.mult)
                rn = cols.tile([128, NS], F32, tag=f"rn")
                nc.vector.tensor_scalar(out=rn[:], in0=rn0[:], scalar1=-TINY,
                                        scalar2=1.0, op0=OP.mult, op1=OP.add)
                nc.vector.tensor_tensor(out=rn[:], in0=rn0[:], in1=rn[:], op=OP.mult)
                a_ = cols.tile([128, NS], F32, tag=f"a")
                nc.vector.tensor_scalar(out=a_[:], in0=rn[:], scalar1=-1.0,
                                        scalar2=2.0, op0=OP.mult, op1=OP.add)
                nc.vector.tensor_tensor(out=a_[:], in0=rn[:], in1=a_[:], op=OP.mult)
                b_ = cols.tile([128, NS], F32, tag=f"b")
                nc.vector.tensor_scalar_add(out=b_[:], in0=rn[:], scalar1=-1.0)
                t2_ = wide.tile([128, NS], F32, tag="mo_a")
                nc.vector.tensor_tensor(out=t2_[:], in0=rn[:], in1=rn0[:], op=OP.mult)
                nc.vector.tensor_tensor(out=t2_[:], in0=t2_[:], in1=rn[:], op=OP.mult)
                nc.vector.tensor_tensor(out=b_[:], in0=t2_[:], in1=b_[:], op=OP.mult)
                nc.vector.tensor_scalar_mul(out=b_[:], in0=b_[:], scalar1=2.0)

                # alpha_b, A coefficients
                alb = cols.tile([128, NS], F32, tag=f"alb")
                nc.vector.tensor_scalar_mul(out=alb[:], in0=rvb[:], scalar1=rdmag[:])
                nc.vector.tensor_tensor(out=alb[:], in0=tvb[:], in1=alb[:], op=OP.subtract)
                bh = cols.tile([128, NS], F32, tag=f"bh")
                nc.vector.tensor_tensor(out=bh[:], in0=b_[:], in1=h[:], op=OP.mult)
                asq = wide.tile([128, NS], F32, tag="mo_a")
                nc.vector.tensor_tensor(out=asq[:], in0=a_[:], in1=a_[:], op=OP.mult)
                A1 = cols.tile([128, NS], F32, tag=f"A1")
                nc.vector.tensor_tensor(out=A1[:], in0=alb[:], in1=asq[:], op=OP.mult)
                A4 = cols.tile([128, NS], F32, tag=f"A4")
                nc.vector.tensor_tensor(out=A4[:], in0=rvb[:], in1=asq[:], op=OP.mult)
                A2 = cols.tile([128, NS], F32, tag=f"A2")
                nc.vector.tensor_tensor(out=A2[:], in0=alb[:], in1=a_[:], op=OP.mult)
                nc.vector.tensor_tensor(out=A2[:], in0=A2[:], in1=bh[:], op=OP.mult)
                nc.vector.tensor_scalar_mul(out=A2[:], in0=A2[:], scalar1=2.0)
                # A3 = alb*bh^2 + rvb*(2ab + (b*n0)^2)
                A3 = cols.tile([128, NS], F32, tag=f"A3")
                bn = wide.tile([128, NS], F32, tag="mo_b")
                nc.vector.tensor_tensor(out=bn[:], in0=b_[:], in1=n0[:], op=OP.mult)
                nc.vector.tensor_tensor(out=bn[:], in0=bn[:], in1=bn[:], op=OP.mult)
                ab = wide.tile([128, NS], F32, tag="mo_c")
                nc.vector.tensor_tensor(out=ab[:], in0=a_[:], in1=b_[:], op=OP.mult)
                nc.vector.scalar_tensor_tensor(out=bn[:], in0=ab[:], scalar=2.0,
                                               in1=bn[:], op0=OP.mult, op1=OP.add)
                nc.vector.tensor_tensor(out=A3[:], in0=rvb[:], in1=bn[:], op=OP.mult)
                bh2 = wide.tile([128, NS], F32, tag="mo_a")
                nc.vector.tensor_tensor(out=bh2[:], in0=bh[:], in1=bh[:], op=OP.mult)
                nc.vector.tensor_tensor(out=bh2[:], in0=alb[:], in1=bh2[:], op=OP.mult)
                nc.vector.tensor_tensor(out=A3[:], in0=A3[:], in1=bh2[:], op=OP.add)

                # ---------------- c = X . p_q   [128, 21*64] ----------------
                c = base.tile([128, 21 * NS], F32, tag="c")
                w1 = base.tile([128, 21 * NS], F32, tag="w1")
                # c[p, q*64+s] = sum_k X[p, k*64+s] * P[k, q]
                GP0 = _ap(pc[:], 0, [[0, NS], [1, 21]])
                GP1 = _ap(pc[:], 21, [[0, NS], [1, 21]])
                GP2 = _ap(pc[:], 42, [[0, NS], [1, 21]])
                X0 = _ap(X[:], 0, [[1, NS], [0, 21]])
                X1 = _ap(X[:], NS, [[1, NS], [0, 21]])
                X2 = _ap(X[:], 2 * NS, [[1, NS], [0, 21]])
                nc.gpsimd.tensor_tensor(out=c[:], in0=X0, in1=GP0, op=OP.mult)
                nc.gpsimd.tensor_tensor(out=w1[:], in0=X1, in1=GP1, op=OP.mult)
                nc.gpsimd.tensor_tensor(out=c[:], in0=c[:], in1=w1[:], op=OP.add)
                nc.gpsimd.tensor_tensor(out=w1[:], in0=X2, in1=GP2, op=OP.mult)
                nc.gpsimd.tensor_tensor(out=c[:], in0=c[:], in1=w1[:], op=OP.add)

                # ---------------- yv0 / y0 ----------------
                # yv0 = (A2*e + A3*c)*c + (A1*esq + A4*w)
                A2b = _ap(A2[:], 0, [[1, NS], [0, 21]])
                A3b = _ap(A3[:], 0, [[1, NS], [0, 21]])
                A1b = _ap(A1[:], 0, [[1, NS], [0, 21]])
                A4b = _ap(A4[:], 0, [[1, NS], [0, 21]])
                ab_ = _ap(a_[:], 0, [[1, NS], [0, 21]])
                e_b = _ap(e21[:], 0, [[0, NS], [1, 21]])
                esq_b = _ap(esq[:], 0, [[0, NS], [1, 21]])
                w_b = _ap(pc[:], 63, [[0, NS], [1, 21]])
                w2 = base.tile([128, 21 * NS], F32, tag="w2")
                nc.vector.tensor_tensor(out=w1[:], in0=A2b, in1=e_b, op=OP.mult)
                nc.vector.tensor_tensor(out=w2[:], in0=A3b, in1=c[:], op=OP.mult)
                nc.vector.tensor_tensor(out=w1[:], in0=w1[:], in1=w2[:], op=OP.add)
                nc.vector.tensor_tensor(out=w1[:], in0=w1[:], in1=c[:], op=OP.mult)
                nc.gpsimd.tensor_tensor(out=w2[:], in0=A1b, in1=esq_b, op=OP.mult)
                nc.vector.tensor_tensor(out=w2[:], in0=w2[:], in1=w1[:], op=OP.add)
                yvb_bg = _ap(yvb[:], 0, [[NF, NS], [1, 21]])
                nc.vector.tensor_tensor(out=w1[:], in0=A4b, in1=w_b, op=OP.mult)
                nc.vector.tensor_tensor(out=yvb_bg, in0=w1[:], in1=w2[:], op=OP.add)
                # y0 = a * c
                yb_bg = _ap(yb[:], 0, [[NF, NS], [1, 21]])
                nc.vector.tensor_tensor(out=yb_bg, in0=ab_, in1=c[:], op=OP.mult)

                # ---------------- angle -> int32 fraction ----------------
                # t = yb*inv2pi ; q = round(t) ; f0 = t - q ; u0 = f0 * 2^31
                tt = base.tile([128, NF * NS], F32, tag="w1")
                nc.vector.tensor_scalar(out=tt[:], in0=yb[:], scalar1=INV2PI,
                                        scalar2=MAGIC_RND, op0=OP.mult, op1=OP.add)
                nc.vector.tensor_scalar(out=tt[:], in0=tt[:], scalar1=MAGIC_RND,
                                        scalar2=None, op0=OP.subtract)
                f0 = base.tile([128, NF * NS], F32, tag="w2")
                nc.vector.scalar_tensor_tensor(out=f0[:], in0=yb[:], scalar=INV2PI,
                                               in1=tt[:], op0=OP.mult, op1=OP.subtract)
                u0 = base.tile([128, NF * NS], I32, tag="c")
                nc.vector.tensor_scalar_mul(out=u0[:], in0=f0[:], scalar1=float(2.0 ** 32))

                # ---------------- per half-tile streaming ----------------
                for hh in range(2):
                    s_h = hh * HALF
                    E = big.tile([128, NL * NF * HALF], F32, tag="E")
                    for j in range(NL):
                        nc.scalar.activation(
                            out=E[:, j * NF * HALF:(j + 1) * NF * HALF],
                            in_=yvb[:, s_h * NF:(s_h + HALF) * NF],
                            func=AF.Exp, scale=float(-0.5 * (4.0 ** j)))

                    for bb_ in range(HALF // BLK):
                        bs = bb_ * BLK
                        # int32 angle cascade for this block: [j*192 + f*8 + s]
                        ub = ubp.tile([128, NL * NF * BLK], I32, tag="ub")
                        nc.vector.tensor_copy(
                            out=ub[:, 0:NF * BLK],
                            in_=u0[:, (s_h + bs) * NF:(s_h + bs + BLK) * NF])
                        for j in range(1, NL):
                            nc.vector.tensor_scalar(
                                out=ub[:, j * NF * BLK:(j + 1) * NF * BLK],
                                in0=ub[:, (j - 1) * NF * BLK:j * NF * BLK],
                                scalar1=1, scalar2=None, op0=OP.logical_shift_left)
                        # Y (sin angles, f32)
                        Yb = yp.tile([128, NL * NF * BLK], F32, tag="Yb")
                        hw_ = NL * NF * BLK // 2
                        nc.gpsimd.tensor_copy(out=Yb[:, 0:hw_], in_=ub[:, 0:hw_])
                        nc.vector.tensor_copy(out=Yb[:, hw_:], in_=ub[:, hw_:])
                        # cos input: |Y|; ACT computes sin2pi(0.25 - |g|)
                        Yc = ycp.tile([128, NL * NF * BLK], F32, tag="yc")
                        nc.vector.tensor_scalar(out=Yc[:].bitcast(U32),
                                                in0=Yb[:].bitcast(U32),
                                                scalar1=0x7FFFFFFF, scalar2=None,
                                                op0=OP.bitwise_and)
                        Sb = sc.tile([128, NL * NF * BLK], F32, tag="sc")
                        nc.scalar.activation(out=Sb[:], in_=Yb[:], func=AF.Arctan,
                                             scale=float(2.0 ** -32))
                        Cb = sc.tile([128, NL * NF * BLK], F32, tag="sc")
                        nc.scalar.activation(out=Cb[:], in_=Yc[:], func=AF.Arctan,
                                             scale=float(-(2.0 ** -32)), bias=quarter[:])

                        ob = outp.tile([128, BLK * FOUT], F32, tag="ob")
                        # layouts: Sb/Cb/ub [j*192 + s*24 + f]; E [j*768 + s*24 + f]
                        o_bg_sin = _ap(ob[:], 96, [[FOUT, BLK], [21, NL], [1, 21]])
                        s_bg = _ap(Sb[:], 0, [[NF, BLK], [NF * BLK, NL], [1, 21]])
                        e_bg = _ap(E[:], bs * NF, [[NF, BLK], [NF * HALF, NL], [1, 21]])
                        nc.vector.tensor_tensor(out=o_bg_sin, in0=s_bg, in1=e_bg, op=OP.mult)
                        # bg cos
                        o_bg_cos = _ap(ob[:], 432, [[FOUT, BLK], [21, NL], [1, 21]])
                        c_bg = _ap(Cb[:], 0, [[NF, BLK], [NF * BLK, NL], [1, 21]])
                        nc.gpsimd.tensor_tensor(out=o_bg_cos, in0=c_bg, in1=e_bg, op=OP.mult)
                        # fg sin: out[s*768 + j*3 + k]
                        o_fg_sin = _ap(ob[:], 0, [[FOUT, BLK], [3, NL], [1, 3]])
                        s_fg = _ap(Sb[:], 21, [[NF, BLK], [NF * BLK, NL], [1, 3]])
                        e_fg = _ap(E[:], bs * NF + 21, [[NF, BLK], [NF * HALF, NL], [1, 3]])
                        nc.vector.tensor_tensor(out=o_fg_sin, in0=s_fg, in1=e_fg, op=OP.mult)
                        # fg cos
                        o_fg_cos = _ap(ob[:], 48, [[FOUT, BLK], [3, NL], [1, 3]])
                        c_fg = _ap(Cb[:], 21, [[NF, BLK], [NF * BLK, NL], [1, 3]])
                        nc.vector.tensor_tensor(out=o_fg_cos, in0=c_fg, in1=e_fg, op=OP.mult)

                        # DMA out
                        oa = out[:, :]
                        nc.sync.dma_start(
                            out=bass.AP(tensor=oa.tensor,
                                        offset=oa.offset + r0 * NS * FOUT + (s_h + bs) * FOUT,
                                        ap=[[NS * FOUT, 128], [1, BLK * FOUT]]),
                            in_=ob[:])

    _split_sync_waits(nc)
    return nc


# ---------------------------------------------------------------------------
# entry point
# ---------------------------------------------------------------------------

_NC_CACHE = []


def kernel(ray_o, ray_d, fg_z_vals, bg_z_vals, radii):
    from concourse.bass_utils import run_bass_kernel_spmd

    if not _NC_CACHE:
        _NC_CACHE.append(build_kernel())
    nc = _NC_CACHE[0]

    pconst = np.concatenate(
        [P_BASIS.reshape(-1), (P_BASIS * P_BASIS).sum(axis=0)]).astype(np.float32)[None, :]

    in_maps = []
    for cidx in range(N_CORES):
        sl = slice(cidx * RAYS_PER_CORE, (cidx + 1) * RAYS_PER_CORE)
        in_maps.append({
            "ray_o": np.ascontiguousarray(ray_o[sl]).astype(np.float32, copy=False),
            "ray_d": np.ascontiguousarray(ray_d[sl]).astype(np.float32, copy=False),
            "fg_z": np.ascontiguousarray(fg_z_vals[sl]).astype(np.float32, copy=False),
            "bg_z": np.ascontiguousarray(bg_z_vals[sl]).astype(np.float32, copy=False),
            "radii": np.ascontiguousarray(radii[sl]).astype(np.float32, copy=False),
            "pconst": pconst,
        })

    res = run_bass_kernel_spmd(nc, in_maps, core_ids=list(range(N_CORES)))
    outs = [res.results[i]["out"].reshape(RAYS_PER_CORE, NS, FOUT)
            for i in range(N_CORES)]
    return np.concatenate(outs, axis=0)



# revision 2
# speedup vs baseline: 1.5802x; 1.5802x over previous
"""MipNerf IPE encoding kernel for Trainium2 (Bass/Tile), 8-core SPMD. v2.

Computes reference(ray_o, ray_d, fg_z_vals, bg_z_vals, radii) -> [2048, 64, 768]:
  fg: diagonal-cov cone cast + diagonal IPE (48 sin + 48 cos features)
  bg: full-cov cone cast + contraction Jacobian + icosahedral-basis IPE
      (336 sin + 336 cos features)

Sharding: embarrassingly data-parallel over rays; 256 rays per core,
2 tiles of 128 rays (partition dim = ray).

v2 design (vs the v1 baseline):
  - Feature axis padded to 26 (21 bg + pad + 3 fg + pad) so every bf16
    tensor_tensor multiply runs with even, 4B-aligned step-1 inner runs
    (2x DVE mode).  Angle/exp/product tiles are laid out [s][j][f26].
  - sin: ACT sin2pi reads the int32 angle tile DIRECTLY (hardware converts
    int32->fp32 before the scale) -- no cast instructions.
  - cos: one fused cast (int32 -> fp16 with scale 2^-32), a 4x-mode 16-bit
    bitwise-AND abs, then ACT sin2pi(0.25 - |g|).
  - angle cascade: binary-doubling int shifts (u[1]=u0<<1, u[2:4]=u[0:2]<<2,
    u[4:8]=u[0:4]<<4, u[8:16]=u[0:8]<<8) -- 4 big 2x-mode ops per quarter.
  - exp: one ACT op per level over the full tile, output bf16.
  - products sin*E / cos*E: bf16 tensor_tensor at 2x, writing a padded
    [s][832] bf16 output block; DMA moves bf16; the host strips padding
    and upcasts to float32 (rel tolerance 2e-2 >> bf16 rounding 4e-3).
"""

import numpy as np

import concourse.bass as bass
import concourse.tile as tile
from concourse import mybir

F32 = mybir.dt.float32
F16 = mybir.dt.float16
BF16 = mybir.dt.bfloat16
I32 = mybir.dt.int32
U16 = mybir.dt.uint16
U32 = mybir.dt.uint32
AF = mybir.ActivationFunctionType
OP = mybir.AluOpType

MAGIC_RND = 12582912.0          # 1.5 * 2^23, float32 round-to-nearest trick
RSQRT_MAGIC = 0x5F3759DF
INV2PI = float(1.0 / (2.0 * np.pi))
TINY = 1e-6

# icosahedral basis (matches reference.py)
P_BASIS = np.array([
    0.8506508, 0.0, 0.5257311, 0.809017, 0.5, 0.309017, 0.5257311, 0.8506508, 0.0,
    1.0, 0.0, 0.0, 0.809017, 0.5, -0.309017, 0.8506508, 0.0, -0.5257311, 0.309017,
    0.809017, -0.5, 0.0, 0.5257311, -0.8506508, 0.5, 0.309017, -0.809017, 0.0, 1.0,
    0.0, -0.5257311, 0.8506508, 0.0, -0.309017, 0.809017, -0.5, 0.0, 0.5257311,
    0.8506508, -0.309017, 0.809017, 0.5, 0.309017, 0.809017, 0.5, 0.5, 0.309017,
    0.809017, 0.5, -0.309017, 0.809017, 0.0, 0.0, 1.0, -0.5, 0.309017, 0.809017,
    -0.809017, 0.5, 0.309017, -0.809017, 0.5, -0.309017], dtype=np.float32).reshape(3, 21)

N_CORES = 8
RAYS_PER_CORE = 256
NS = 64           # samples per ray
NL = 16           # frequency levels
NF = 26           # padded features: 0..20 bg, 21 pad, 22..24 fg, 25 pad
NQ = 16           # samples per quarter
NO = 8            # samples per octant
OBW = 832         # padded out width per sample: 64 fgs + 64 fgc + 352 bgs + 352 bgc
FOUT = 768


# ---------------------------------------------------------------------------
# walrus workarounds (same as v1)
# ---------------------------------------------------------------------------

_PATCHED = False


def _apply_patches():
    """1) split >1 sem-waits per instruction (this walrus rejects multi-wait
    instructions);  2) rewrite sentinel Arctan activations into Sin2pi."""
    global _PATCHED
    if _PATCHED:
        return
    _PATCHED = True

    import concourse.bass2jax as bass2jax

    orig_compile = bass2jax.compile_bir_kernel

    def patched_compile(bir_json, tmpdir, neff_name="file.neff"):
        if isinstance(bir_json, bytes):
            bir_json = bir_json.replace(b'"func":"Arctan"', b'"func":"Sin2pi"')
        else:
            bir_json = bir_json.replace('"func":"Arctan"', '"func":"Sin2pi"')
        return orig_compile(bir_json, tmpdir, neff_name=neff_name)

    bass2jax.compile_bir_kernel = patched_compile


_waitsplit_ctr = [0]


def _split_sync_waits(nc, max_waits=1):
    n_split = 0
    for fn in nc.m.functions:
        for bb in fn.blocks:
            il = bb.instructions
            i = 0
            while i < len(il):
                ins = il[i]
                si = ins.sync_info
                waits = list(si.on_wait) if si is not None else []
                if len(waits) > max_waits:
                    extra, keep = waits[:-max_waits], waits[-max_waits:]
                    pos = i
                    for j in range(0, len(extra), max_waits):
                        chunk = extra[j:j + max_waits]
                        _waitsplit_ctr[0] += 1
                        nop = mybir.InstNoOp(
                            name=f"waitsplit_{_waitsplit_ctr[0]}", ins=[], outs=[])
                        nop.engine = ins.engine
                        nop.sync_info = mybir.SyncInfo(on_wait=chunk, on_update=[])
                        nc.register_instruction(nop, overwrite=True)
                        il.insert(pos, nop)
                        pos += 1
                        i += 1
                    ins.sync_info = mybir.SyncInfo(
                        on_wait=keep, on_update=list(si.on_update))
                    n_split += 1
                i += 1
    return n_split


# ---------------------------------------------------------------------------
# AP helpers
# ---------------------------------------------------------------------------

def _ap(base, offset_elems, dims):
    """Custom AP over a tile/AP: keep partition dim, replace free dims."""
    return bass.AP(tensor=base.tensor, offset=base.offset + offset_elems,
                   ap=[base.ap[0]] + [list(d) for d in dims])


# ---------------------------------------------------------------------------
# kernel body
# ---------------------------------------------------------------------------

def _moments(nc, wide, z, r2, out_tm2, out_tv, out_rv):
    """Frustum moments from z [128, 65] -> t_mean2 (=2*t_mean), t_var, r_var
    [128, 64].  r2 = radii^2 per-ray [128, 1]."""
    t0 = z[:, 0:NS]
    t1 = z[:, 1:NS + 1]
    sm = wide.tile([128, NS], F32, tag="mo_a")
    nc.vector.tensor_tensor(out=sm[:], in0=t0, in1=t1, op=OP.add)
    df = wide.tile([128, NS], F32, tag="mo_b")
    nc.vector.tensor_tensor(out=df[:], in0=t1, in1=t0, op=OP.subtract)
    sm2 = wide.tile([128, NS], F32, tag="mo_c")
    nc.vector.tensor_tensor(out=sm2[:], in0=sm[:], in1=sm[:], op=OP.mult)
    df2 = wide.tile([128, NS], F32, tag="mo_d")
    nc.vector.tensor_tensor(out=df2[:], in0=df[:], in1=df[:], op=OP.mult)
    # denom4 = 3*sm2 + df2
    den4 = wide.tile([128, NS], F32, tag="mo_e")
    nc.vector.scalar_tensor_tensor(out=den4[:], in0=sm2[:], scalar=3.0,
                                   in1=df2[:], op0=OP.mult, op1=OP.add)
    rden4 = wide.tile([128, NS], F32, tag="mo_f")
    nc.vector.reciprocal(out=rden4[:], in_=den4[:])
    u1 = wide.tile([128, NS], F32, tag="mo_g")
    nc.vector.tensor_tensor(out=u1[:], in0=df2[:], in1=rden4[:], op=OP.mult)
    # t_mean2 = sm * (1 + 2*u1)
    tmp = wide.tile([128, NS], F32, tag="mo_h")
    nc.vector.tensor_scalar(out=tmp[:], in0=u1[:], scalar1=2.0, scalar2=1.0,
                            op0=OP.mult, op1=OP.add)
    nc.vector.tensor_tensor(out=out_tm2[:], in0=sm[:], in1=tmp[:], op=OP.mult)
    # t_var = df2/12 - (4/15) * u1^2 * (den4 - 1.25*df2)
    u1sq = wide.tile([128, NS], F32, tag="mo_h")
    nc.vector.tensor_tensor(out=u1sq[:], in0=u1[:], in1=u1[:], op=OP.mult)
    g2 = wide.tile([128, NS], F32, tag="mo_a")
    nc.vector.scalar_tensor_tensor(out=g2[:], in0=df2[:], scalar=-1.25,
                                   in1=den4[:], op0=OP.mult, op1=OP.add)
    g3 = wide.tile([128, NS], F32, tag="mo_c")
    nc.vector.tensor_tensor(out=g3[:], in0=u1sq[:], in1=g2[:], op=OP.mult)
    g5 = wide.tile([128, NS], F32, tag="mo_e")
    nc.vector.tensor_scalar_mul(out=g5[:], in0=df2[:], scalar1=float(1.0 / 12.0))
    nc.vector.scalar_tensor_tensor(out=out_tv[:], in0=g3[:], scalar=float(-4.0 / 15.0),
                                   in1=g5[:], op0=OP.mult, op1=OP.add)
    # r_var = r2 * (sm2/16 + (5/48)*df2 - (1/15)*u1*df2)
    h1 = wide.tile([128, NS], F32, tag="mo_a")
    nc.vector.tensor_tensor(out=h1[:], in0=u1[:], in1=df2[:], op=OP.mult)
    h2 = wide.tile([128, NS], F32, tag="mo_c")
    nc.vector.tensor_scalar_mul(out=h2[:], in0=sm2[:], scalar1=float(1.0 / 16.0))
    h4 = wide.tile([128, NS], F32, tag="mo_e")
    nc.vector.scalar_tensor_tensor(out=h4[:], in0=df2[:], scalar=float(5.0 / 48.0),
                                   in1=h2[:], op0=OP.mult, op1=OP.add)
    h5 = wide.tile([128, NS], F32, tag="mo_a")
    nc.vector.scalar_tensor_tensor(out=h5[:], in0=h1[:], scalar=float(-1.0 / 15.0),
                                   in1=h4[:], op0=OP.mult, op1=OP.add)
    nc.vector.tensor_scalar_mul(out=out_rv[:], in0=h5[:], scalar1=r2[:])


def build_kernel():
    """Build the 8-core SPMD Bass module (per-core: 256 rays)."""
    _apply_patches()
    nc = bass.Bass(dynamic_dma_scratch_size=4096)

    ray_o = nc.dram_tensor("ray_o", [RAYS_PER_CORE, 3], F32, kind="ExternalInput")
    ray_d = nc.dram_tensor("ray_d", [RAYS_PER_CORE, 3], F32, kind="ExternalInput")
    fg_z = nc.dram_tensor("fg_z", [RAYS_PER_CORE, NS + 1], F32, kind="ExternalInput")
    bg_z = nc.dram_tensor("bg_z", [RAYS_PER_CORE, NS + 1], F32, kind="ExternalInput")
    radii = nc.dram_tensor("radii", [RAYS_PER_CORE, 1], F32, kind="ExternalInput")
    pconst = nc.dram_tensor("pconst", [1, 84], F32, kind="ExternalInput")
    out = nc.dram_tensor("out", [RAYS_PER_CORE, NS * OBW], BF16, kind="ExternalOutput")

    with tile.TileContext(nc) as tc:
        import contextlib
        ctx = contextlib.ExitStack()
        with ctx:
            consts = ctx.enter_context(tc.tile_pool(name="consts", bufs=1))
            base = ctx.enter_context(tc.tile_pool(name="base", bufs=1))
            wide = ctx.enter_context(tc.tile_pool(name="wide", bufs=2))
            upool = ctx.enter_context(tc.tile_pool(name="upool", bufs=1))
            uhpool = ctx.enter_context(tc.tile_pool(name="uhpool", bufs=2))
            spool = ctx.enter_context(tc.tile_pool(name="spool", bufs=2))
            cpool = ctx.enter_context(tc.tile_pool(name="cpool", bufs=2))
            epool = ctx.enter_context(tc.tile_pool(name="epool", bufs=1))
            obpool = ctx.enter_context(tc.tile_pool(name="obpool", bufs=2))

            # constants
            pc = consts.tile([128, 84], F32)
            pca = pconst[:, :]
            nc.sync.dma_start(out=pc[:], in_=bass.AP(
                tensor=pca.tensor, offset=pca.offset, ap=[[0, 128], [1, 84]]))
            magic_u = consts.tile([128, 1], U32)
            nc.vector.memset(magic_u, RSQRT_MAGIC)
            quarter = consts.tile([128, 1], F32)
            nc.vector.memset(quarter, 0.25)

            for t in range(2):
                r0 = t * 128

                # ---------------- load inputs ----------------
                zf = base.tile([128, NS + 1], F32, tag="zf")
                nc.sync.dma_start(out=zf[:], in_=fg_z[r0:r0 + 128, :])
                zb = base.tile([128, NS + 1], F32, tag="zb")
                nc.sync.dma_start(out=zb[:], in_=bg_z[r0:r0 + 128, :])
                o3 = base.tile([128, 3], F32, tag="o3")
                nc.sync.dma_start(out=o3[:], in_=ray_o[r0:r0 + 128, :])
                d3 = base.tile([128, 3], F32, tag="d3")
                nc.sync.dma_start(out=d3[:], in_=ray_d[r0:r0 + 128, :])
                rad = base.tile([128, 1], F32, tag="rad")
                nc.sync.dma_start(out=rad[:], in_=radii[r0:r0 + 128, :])

                # ---------------- per-ray scalars ----------------
                r2 = base.tile([128, 1], F32, tag="r2")
                nc.vector.tensor_tensor(out=r2[:], in0=rad[:], in1=rad[:], op=OP.mult)
                dk2 = base.tile([128, 3], F32, tag="dk2")
                nc.vector.tensor_tensor(out=dk2[:], in0=d3[:], in1=d3[:], op=OP.mult)
                dmag = base.tile([128, 1], F32, tag="dmag")
                nc.vector.tensor_tensor(out=dmag[:], in0=dk2[:, 0:1], in1=dk2[:, 1:2], op=OP.add)
                nc.vector.tensor_tensor(out=dmag[:], in0=dmag[:], in1=dk2[:, 2:3], op=OP.add)
                nc.vector.tensor_scalar_max(out=dmag[:], in0=dmag[:], scalar1=1e-8)
                rdmag = base.tile([128, 1], F32, tag="rdmag")
                nc.vector.reciprocal(out=rdmag[:], in_=dmag[:])
                hd3 = base.tile([128, 3], F32, tag="hd3")
                nc.vector.tensor_scalar_mul(out=hd3[:], in0=d3[:], scalar1=0.5)
                # inv2pi-folded copies for the fg angle path
                hd3i = base.tile([128, 3], F32, tag="hd3i")
                nc.vector.tensor_scalar_mul(out=hd3i[:], in0=hd3[:], scalar1=INV2PI)
                o3i = base.tile([128, 3], F32, tag="o3i")
                nc.vector.tensor_scalar_mul(out=o3i[:], in0=o3[:], scalar1=INV2PI)

                # e = d @ P  [128, 21], esq
                e21 = base.tile([128, 21], F32, tag="e21")
                nc.vector.tensor_scalar_mul(out=e21[:], in0=pc[:, 0:21], scalar1=d3[:, 0:1])
                tmp21 = base.tile([128, 21], F32, tag="tmp21")
                nc.vector.tensor_scalar_mul(out=tmp21[:], in0=pc[:, 21:42], scalar1=d3[:, 1:2])
                nc.vector.tensor_tensor(out=e21[:], in0=e21[:], in1=tmp21[:], op=OP.add)
                nc.vector.tensor_scalar_mul(out=tmp21[:], in0=pc[:, 42:63], scalar1=d3[:, 2:3])
                nc.vector.tensor_tensor(out=e21[:], in0=e21[:], in1=tmp21[:], op=OP.add)
                esq = base.tile([128, 21], F32, tag="esq")
                nc.vector.tensor_tensor(out=esq[:], in0=e21[:], in1=e21[:], op=OP.mult)

                # ---------------- moments ----------------
                tm2f = base.tile([128, NS], F32, tag="tm2f")
                tvf = base.tile([128, NS], F32, tag="tvf")
                rvf = base.tile([128, NS], F32, tag="rvf")
                _moments(nc, wide, zf, r2, tm2f, tvf, rvf)
                tm2b = base.tile([128, NS], F32, tag="tm2b")
                tvb = base.tile([128, NS], F32, tag="tvb")
                rvb = base.tile([128, NS], F32, tag="rvb")
                _moments(nc, wide, zb, r2, tm2b, tvb, rvb)

                # g0 (angle/2pi) and yv0 (variance) tiles, [s(64)][f(26)]
                g0 = base.tile([128, NS * NF], F32, tag="g0")
                nc.vector.memset(g0, 0.0)
                yv0 = base.tile([128, NS * NF], F32, tag="yv0")
                nc.vector.memset(yv0, 0.0)

                # ---------------- fg: mean + cov_diag into cols 22..24 -------
                alf = wide.tile([128, NS], F32, tag="mo_b")
                nc.vector.tensor_scalar_mul(out=alf[:], in0=rvf[:], scalar1=rdmag[:])
                nc.vector.tensor_tensor(out=alf[:], in0=tvf[:], in1=alf[:], op=OP.subtract)
                for k in range(3):
                    # g0_fg = (tm2f * hd3_k + o_k) * inv2pi (folded constants)
                    nc.vector.tensor_scalar(
                        out=_ap(g0[:], 22 + k, [[NF, NS]]), in0=tm2f[:],
                        scalar1=hd3i[:, k:k + 1], scalar2=o3i[:, k:k + 1],
                        op0=OP.mult, op1=OP.add)
                    # cd_k = alf * dk2_k + rvf
                    nc.vector.scalar_tensor_tensor(
                        out=_ap(yv0[:], 22 + k, [[NF, NS]]), in0=alf[:],
                        scalar=dk2[:, k:k + 1], in1=rvf[:], op0=OP.mult, op1=OP.add)

                # ---------------- bg: contraction scalars ----------------
                X = base.tile([128, 3 * NS], F32, tag="X")          # [k*64+s]
                for k in range(3):
                    nc.vector.tensor_scalar(
                        out=X[:, k * NS:(k + 1) * NS], in0=tm2b[:],
                        scalar1=hd3[:, k:k + 1], scalar2=o3[:, k:k + 1],
                        op0=OP.mult, op1=OP.add)
                s2 = base.tile([128, NS], F32, tag="s2")
                nc.vector.tensor_tensor(out=s2[:], in0=X[:, 0:NS], in1=X[:, 0:NS], op=OP.mult)
                w0 = wide.tile([128, NS], F32, tag="mo_a")
                nc.vector.tensor_tensor(out=w0[:], in0=X[:, NS:2 * NS], in1=X[:, NS:2 * NS], op=OP.mult)
                nc.vector.tensor_tensor(out=s2[:], in0=s2[:], in1=w0[:], op=OP.add)
                nc.vector.tensor_tensor(out=w0[:], in0=X[:, 2 * NS:3 * NS], in1=X[:, 2 * NS:3 * NS], op=OP.mult)
                nc.vector.tensor_tensor(out=s2[:], in0=s2[:], in1=w0[:], op=OP.add)
                # h = d . X
                h = base.tile([128, NS], F32, tag="h")
                nc.vector.tensor_scalar_mul(out=h[:], in0=X[:, 0:NS], scalar1=d3[:, 0:1])
                nc.vector.scalar_tensor_tensor(out=h[:], in0=X[:, NS:2 * NS],
                                               scalar=d3[:, 1:2], in1=h[:],
                                               op0=OP.mult, op1=OP.add)
                nc.vector.scalar_tensor_tensor(out=h[:], in0=X[:, 2 * NS:3 * NS],
                                               scalar=d3[:, 2:3], in1=h[:],
                                               op0=OP.mult, op1=OP.add)

                # rsqrt(s2): magic seed + 4 Newton iterations
                rn0 = base.tile([128, NS], F32, tag="rn0")
                seed_u = wide.tile([128, NS], U32, tag="mo_a")
                nc.vector.tensor_scalar(out=seed_u[:], in0=s2[:].bitcast(U32),
                                        scalar1=1, scalar2=None,
                                        op0=OP.logical_shift_right)
                nc.vector.tensor_tensor(
                    out=rn0[:].bitcast(U32),
                    in0=_ap(magic_u[:], 0, [[0, NS]]),
                    in1=seed_u[:], op=OP.subtract)
                for _ in range(4):
                    nr = wide.tile([128, NS], F32, tag="mo_b")
                    nc.vector.tensor_tensor(out=nr[:], in0=s2[:], in1=rn0[:], op=OP.mult)
                    nc.vector.tensor_tensor(out=nr[:], in0=nr[:], in1=rn0[:], op=OP.mult)
                    nc.vector.tensor_scalar(out=nr[:], in0=nr[:], scalar1=-0.5,
                                            scalar2=1.5, op0=OP.mult, op1=OP.add)
                    nc.vector.tensor_tensor(out=rn0[:], in0=rn0[:], in1=nr[:], op=OP.mult)

                n0 = base.tile([128, NS], F32, tag="n0")
                nc.vector.tensor_tensor(out=n0[:], in0=s2[:], in1=rn0[:], op=OP.mult)
                rn = base.tile([128, NS], F32, tag="rn")
                nc.vector.tensor_scalar(out=rn[:], in0=rn0[:], scalar1=-TINY,
                                        scalar2=1.0, op0=OP.mult, op1=OP.add)
                nc.vector.tensor_tensor(out=rn[:], in0=rn0[:], in1=rn[:], op=OP.mult)
                a_ = base.tile([128, NS], F32, tag="a")
                nc.vector.tensor_scalar(out=a_[:], in0=rn[:], scalar1=-1.0,
                                        scalar2=2.0, op0=OP.mult, op1=OP.add)
                nc.vector.tensor_tensor(out=a_[:], in0=rn[:], in1=a_[:], op=OP.mult)
                b_ = base.tile([128, NS], F32, tag="b")
                nc.vector.tensor_scalar_add(out=b_[:], in0=rn[:], scalar1=-1.0)
                t2_ = wide.tile([128, NS], F32, tag="mo_a")
                nc.vector.tensor_tensor(out=t2_[:], in0=rn[:], in1=rn0[:], op=OP.mult)
                nc.vector.tensor_tensor(out=t2_[:], in0=t2_[:], in1=rn[:], op=OP.mult)
                nc.vector.tensor_tensor(out=b_[:], in0=t2_[:], in1=b_[:], op=OP.mult)
                nc.vector.tensor_scalar_mul(out=b_[:], in0=b_[:], scalar1=2.0)

                # alpha_b, A coefficients
                alb = base.tile([128, NS], F32, tag="alb")
                nc.vector.tensor_scalar_mul(out=alb[:], in0=rvb[:], scalar1=rdmag[:])
                nc.vector.tensor_tensor(out=alb[:], in0=tvb[:], in1=alb[:], op=OP.subtract)
                bh = base.tile([128, NS], F32, tag="bh")
                nc.vector.tensor_tensor(out=bh[:], in0=b_[:], in1=h[:], op=OP.mult)
                asq = wide.tile([128, NS], F32, tag="mo_a")
                nc.vector.tensor_tensor(out=asq[:], in0=a_[:], in1=a_[:], op=OP.mult)
                A1 = base.tile([128, NS], F32, tag="A1")
                nc.vector.tensor_tensor(out=A1[:], in0=alb[:], in1=asq[:], op=OP.mult)
                A4 = base.tile([128, NS], F32, tag="A4")
                nc.vector.tensor_tensor(out=A4[:], in0=rvb[:], in1=asq[:], op=OP.mult)
                A2 = base.tile([128, NS], F32, tag="A2")
                nc.vector.tensor_tensor(out=A2[:], in0=alb[:], in1=a_[:], op=OP.mult)
                nc.vector.tensor_tensor(out=A2[:], in0=A2[:], in1=bh[:], op=OP.mult)
                nc.vector.tensor_scalar_mul(out=A2[:], in0=A2[:], scalar1=2.0)
                # A3 = alb*bh^2 + rvb*(2ab + (b*n0)^2)
                A3 = base.tile([128, NS], F32, tag="A3")
                bn = wide.tile([128, NS], F32, tag="mo_b")
                nc.vector.tensor_tensor(out=bn[:], in0=b_[:], in1=n0[:], op=OP.mult)
                nc.vector.tensor_tensor(out=bn[:], in0=bn[:], in1=bn[:], op=OP.mult)
                ab = wide.tile([128, NS], F32, tag="mo_c")
                nc.vector.tensor_tensor(out=ab[:], in0=a_[:], in1=b_[:], op=OP.mult)
                nc.vector.scalar_tensor_tensor(out=bn[:], in0=ab[:], scalar=2.0,
                                               in1=bn[:], op0=OP.mult, op1=OP.add)
                nc.vector.tensor_tensor(out=A3[:], in0=rvb[:], in1=bn[:], op=OP.mult)
                bh2 = wide.tile([128, NS], F32, tag="mo_a")
                nc.vector.tensor_tensor(out=bh2[:], in0=bh[:], in1=bh[:], op=OP.mult)
                nc.vector.tensor_tensor(out=bh2[:], in0=alb[:], in1=bh2[:], op=OP.mult)
                nc.vector.tensor_tensor(out=A3[:], in0=A3[:], in1=bh2[:], op=OP.add)
                # ai = a * inv2pi (fold the angle scale into a)
                ai = base.tile([128, NS], F32, tag="ai")
                nc.vector.tensor_scalar_mul(out=ai[:], in0=a_[:], scalar1=INV2PI)

                # ---------------- c = X . p_q   [s*21+q] (s-major) ----------
                c = base.tile([128, 21 * NS], F32, tag="c")
                w1 = base.tile([128, NS * NF], F32, tag="w1")  # scratch (1664)
                w2 = base.tile([128, 21 * NS], F32, tag="w2")
                GP0 = _ap(pc[:], 0, [[0, NS], [1, 21]])
                GP1 = _ap(pc[:], 21, [[0, NS], [1, 21]])
                GP2 = _ap(pc[:], 42, [[0, NS], [1, 21]])
                X0 = _ap(X[:], 0, [[1, NS], [0, 21]])
                X1 = _ap(X[:], NS, [[1, NS], [0, 21]])
                X2 = _ap(X[:], 2 * NS, [[1, NS], [0, 21]])
                w1c = _ap(w1[:], 0, [[1, 21 * NS]])
                nc.gpsimd.tensor_tensor(out=c[:], in0=X0, in1=GP0, op=OP.mult)
                nc.gpsimd.tensor_tensor(out=w1c, in0=X1, in1=GP1, op=OP.mult)
                nc.gpsimd.tensor_tensor(out=c[:], in0=c[:], in1=w1c, op=OP.add)
                nc.gpsimd.tensor_tensor(out=w1c, in0=X2, in1=GP2, op=OP.mult)
                nc.gpsimd.tensor_tensor(out=c[:], in0=c[:], in1=w1c, op=OP.add)

                # ---------------- yv0 / g0 bg columns ----------------
                # yv0_q = (A2*e + A3*c)*c + (A1*esq + A4*w);  g0_q = ai*c
                A2b = _ap(A2[:], 0, [[1, NS], [0, 21]])
                A3b = _ap(A3[:], 0, [[1, NS], [0, 21]])
                A1b = _ap(A1[:], 0, [[1, NS], [0, 21]])
                A4b = _ap(A4[:], 0, [[1, NS], [0, 21]])
                aib = _ap(ai[:], 0, [[1, NS], [0, 21]])
                e_b = _ap(e21[:], 0, [[0, NS], [1, 21]])
                esq_b = _ap(esq[:], 0, [[0, NS], [1, 21]])
                w_b = _ap(pc[:], 63, [[0, NS], [1, 21]])
                w2c = _ap(w2[:], 0, [[1, 21 * NS]])
                nc.gpsimd.tensor_tensor(out=w1c, in0=A2b, in1=e_b, op=OP.mult)
                nc.gpsimd.tensor_tensor(out=w2c, in0=A3b, in1=c[:], op=OP.mult)
                nc.vector.tensor_tensor(out=w1c, in0=w1c, in1=w2c, op=OP.add)
                nc.vector.tensor_tensor(out=w1c, in0=w1c, in1=c[:], op=OP.mult)
                nc.gpsimd.tensor_tensor(out=w2c, in0=A1b, in1=esq_b, op=OP.mult)
                nc.vector.tensor_tensor(out=w2c, in0=w2c, in1=w1c, op=OP.add)
                yv_bg = _ap(yv0[:], 0, [[NF, NS], [1, 21]])
                nc.gpsimd.tensor_tensor(out=w1c, in0=A4b, in1=w_b, op=OP.mult)
                nc.vector.tensor_tensor(out=yv_bg, in0=w1c, in1=w2c, op=OP.add)
                # g0 bg = ai * c  (strided out into [s][f] cols 0..20)
                g0_bg = _ap(g0[:], 0, [[NF, NS], [1, 21]])
                nc.gpsimd.tensor_tensor(out=g0_bg, in0=aib, in1=c[:], op=OP.mult)

                # ---------------- frac + int angle base ----------------
                # q = round(g0); f0 = g0 - q; u0 = f0 * 2^32 (int32)
                qr = base.tile([128, NS * NF], F32, tag="qr")
                nc.vector.tensor_scalar(out=qr[:], in0=g0[:], scalar1=MAGIC_RND,
                                        scalar2=MAGIC_RND, op0=OP.add, op1=OP.subtract)
                f0 = base.tile([128, NS * NF], F32, tag="f0")
                nc.vector.tensor_tensor(out=f0[:], in0=g0[:], in1=qr[:], op=OP.subtract)

                # ---------------- exp levels: E[s][j][f] bf16 ----------------
                E = epool.tile([128, NS * NL * NF], BF16, tag="E")
                for j in range(NL):
                    nc.scalar.activation(
                        out=_ap(E[:], j * NF, [[NL * NF, NS], [1, NF]]),
                        in_=_ap(yv0[:], 0, [[NF, NS], [1, NF]]),
                        func=AF.Exp, scale=float(-0.5 * (4.0 ** j)))

                # ---------------- per-quarter angle cascade + per-octant -----
                for qq in range(4):
                    s_q = qq * NQ
                    uq = upool.tile([128, NQ * NL * NF], I32, tag="uq")
                    # u0 into level-0 slots
                    nc.vector.tensor_scalar_mul(
                        out=_ap(uq[:], 0, [[NL * NF, NQ], [1, NF]]),
                        in0=f0[:, s_q * NF:(s_q + NQ) * NF],
                        scalar1=float(2.0 ** 32))
                    # binary-doubling cascade: [1]=[0]<<1; [2:4]=[0:2]<<2;
                    # [4:8]=[0:4]<<4; [8:16]=[0:8]<<8
                    for (src, w, sh) in ((0, 1, 1), (0, 2, 2), (0, 4, 4), (0, 8, 8)):
                        nc.vector.tensor_scalar(
                            out=_ap(uq[:], w * NF, [[NL * NF, NQ], [1, w * NF]]),
                            in0=_ap(uq[:], src * NF, [[NL * NF, NQ], [1, w * NF]]),
                            scalar1=sh, scalar2=None, op0=OP.logical_shift_left)

                    for oo in range(2):
                        s_o = s_q + oo * NO         # absolute first sample
                        uo = uq[:, oo * NO * NL * NF:(oo + 1) * NO * NL * NF]
                        # sin: ACT reads int32 directly
                        S = spool.tile([128, NO * NL * NF], BF16, tag="S")
                        nc.scalar.activation(out=S[:], in_=uo, func=AF.Arctan,
                                             scale=float(2.0 ** -32))
                        # cos: fused cast to fp16 angle, 4x abs, ACT
                        uh = uhpool.tile([128, NO * NL * NF], F16, tag="uh")
                        nc.vector.tensor_scalar_mul(out=uh[:], in0=uo,
                                                    scalar1=float(2.0 ** -32))
                        nc.vector.tensor_scalar(out=uh[:].bitcast(U16), in0=uh[:].bitcast(U16),
                                                scalar1=0x7FFF, scalar2=None,
                                                op0=OP.bitwise_and)
                        C = cpool.tile([128, NO * NL * NF], BF16, tag="C")
                        nc.scalar.activation(out=C[:], in_=uh[:], func=AF.Arctan,
                                             scale=-1.0, bias=quarter[:])

                        # products into padded out block [s][832]
                        ob = obpool.tile([128, NO * OBW], BF16, tag="ob")
                        eoff = s_o * NL * NF
                        # bg sin: f 0..21 (run 22)
                        nc.vector.tensor_tensor(
                            out=_ap(ob[:], 128, [[OBW, NO], [22, NL], [1, 22]]),
                            in0=_ap(S[:], 0, [[NL * NF, NO], [NF, NL], [1, 22]]),
                            in1=_ap(E[:], eoff, [[NL * NF, NO], [NF, NL], [1, 22]]),
                            op=OP.mult)
                        # bg cos
                        nc.vector.tensor_tensor(
                            out=_ap(ob[:], 480, [[OBW, NO], [22, NL], [1, 22]]),
                            in0=_ap(C[:], 0, [[NL * NF, NO], [NF, NL], [1, 22]]),
                            in1=_ap(E[:], eoff, [[NL * NF, NO], [NF, NL], [1, 22]]),
                            op=OP.mult)
                        # fg sin: f 22..25 (run 4)
                        nc.vector.tensor_tensor(
                            out=_ap(ob[:], 0, [[OBW, NO], [4, NL], [1, 4]]),
                            in0=_ap(S[:], 22, [[NL * NF, NO], [NF, NL], [1, 4]]),
                            in1=_ap(E[:], eoff + 22, [[NL * NF, NO], [NF, NL], [1, 4]]),
                            op=OP.mult)
                        # fg cos
                        nc.vector.tensor_tensor(
                            out=_ap(ob[:], 64, [[OBW, NO], [4, NL], [1, 4]]),
                            in0=_ap(C[:], 22, [[NL * NF, NO], [NF, NL], [1, 4]]),
                            in1=_ap(E[:], eoff + 22, [[NL * NF, NO], [NF, NL], [1, 4]]),
                            op=OP.mult)

                        # DMA out
                        oa = out[:, :]
                        nc.sync.dma_start(
                            out=bass.AP(tensor=oa.tensor,
                                        offset=oa.offset + r0 * NS * OBW + s_o * OBW,
                                        ap=[[NS * OBW, 128], [1, NO * OBW]]),
                            in_=ob[:])

    _split_sync_waits(nc)
    return nc


# ---------------------------------------------------------------------------
# entry point
# ---------------------------------------------------------------------------

_NC_CACHE = []


def kernel(ray_o, ray_d, fg_z_vals, bg_z_vals, radii):
    from concourse.bass_utils import run_bass_kernel_spmd

    if not _NC_CACHE:
        _NC_CACHE.append(build_kernel())
    nc = _NC_CACHE[0]

    pconst = np.concatenate(
        [P_BASIS.reshape(-1), (P_BASIS * P_BASIS).sum(axis=0)]).astype(np.float32)[None, :]

    in_maps = []
    for cidx in range(N_CORES):
        sl = slice(cidx * RAYS_PER_CORE, (cidx + 1) * RAYS_PER_CORE)
        in_maps.append({
            "ray_o": np.ascontiguousarray(ray_o[sl]).astype(np.float32, copy=False),
            "ray_d": np.ascontiguousarray(ray_d[sl]).astype(np.float32, copy=False),
            "fg_z": np.ascontiguousarray(fg_z_vals[sl]).astype(np.float32, copy=False),
            "bg_z": np.ascontiguousarray(bg_z_vals[sl]).astype(np.float32, copy=False),
            "radii": np.ascontiguousarray(radii[sl]).astype(np.float32, copy=False),
            "pconst": pconst,
        })

    res = run_bass_kernel_spmd(nc, in_maps, core_ids=list(range(N_CORES)))
    full = np.concatenate(
        [np.asarray(res.results[i]["out"]) for i in range(N_CORES)], axis=0)
    v = full.reshape(2048, NS, OBW).astype(np.float32)
    fs = v[:, :, 0:64].reshape(2048, NS, 16, 4)[:, :, :, :3].reshape(2048, NS, 48)
    fc = v[:, :, 64:128].reshape(2048, NS, 16, 4)[:, :, :, :3].reshape(2048, NS, 48)
    bs = v[:, :, 128:480].reshape(2048, NS, 16, 22)[:, :, :, :21].reshape(2048, NS, 336)
    bc = v[:, :, 480:832].reshape(2048, NS, 16, 22)[:, :, :, :21].reshape(2048, NS, 336)
    return np.ascontiguousarray(
        np.concatenate([fs, fc, bs, bc], axis=-1), dtype=np.float32)


# revision 13
# speedup vs baseline: 1.6193x; 1.0247x over previous
"""MipNerf IPE encoding kernel for Trainium2 (Bass/Tile), 8-core SPMD. v2.

Computes reference(ray_o, ray_d, fg_z_vals, bg_z_vals, radii) -> [2048, 64, 768]:
  fg: diagonal-cov cone cast + diagonal IPE (48 sin + 48 cos features)
  bg: full-cov cone cast + contraction Jacobian + icosahedral-basis IPE
      (336 sin + 336 cos features)

Sharding: embarrassingly data-parallel over rays; 256 rays per core,
2 tiles of 128 rays (partition dim = ray).

v2 design (vs the v1 baseline):
  - Feature axis padded to 26 (21 bg + pad + 3 fg + pad) so every bf16
    tensor_tensor multiply runs with even, 4B-aligned step-1 inner runs
    (2x DVE mode).  Angle/exp/product tiles are laid out [s][j][f26].
  - sin: ACT sin2pi reads the int32 angle tile DIRECTLY (hardware converts
    int32->fp32 before the scale) -- no cast instructions.
  - cos: one fused cast (int32 -> fp16 with scale 2^-32), a 4x-mode 16-bit
    bitwise-AND abs, then ACT sin2pi(0.25 - |g|).
  - angle cascade: binary-doubling int shifts (u[1]=u0<<1, u[2:4]=u[0:2]<<2,
    u[4:8]=u[0:4]<<4, u[8:16]=u[0:8]<<8) -- 4 big 2x-mode ops per quarter.
  - exp: one ACT op per level over the full tile, output bf16.
  - products sin*E / cos*E: bf16 tensor_tensor at 2x, writing a padded
    [s][832] bf16 output block; DMA moves bf16; the host strips padding
    and upcasts to float32 (rel tolerance 2e-2 >> bf16 rounding 4e-3).
"""

import numpy as np

import concourse.bass as bass
import concourse.tile as tile
from concourse import mybir

F32 = mybir.dt.float32
F16 = mybir.dt.float16
BF16 = mybir.dt.bfloat16
I32 = mybir.dt.int32
U16 = mybir.dt.uint16
U32 = mybir.dt.uint32
AF = mybir.ActivationFunctionType
OP = mybir.AluOpType

MAGIC_RND = 12582912.0          # 1.5 * 2^23, float32 round-to-nearest trick
RSQRT_MAGIC = 0x5F3759DF
INV2PI = float(1.0 / (2.0 * np.pi))
TINY = 1e-6

# icosahedral basis (matches reference.py)
P_BASIS = np.array([
    0.8506508, 0.0, 0.5257311, 0.809017, 0.5, 0.309017, 0.5257311, 0.8506508, 0.0,
    1.0, 0.0, 0.0, 0.809017, 0.5, -0.309017, 0.8506508, 0.0, -0.5257311, 0.309017,
    0.809017, -0.5, 0.0, 0.5257311, -0.8506508, 0.5, 0.309017, -0.809017, 0.0, 1.0,
    0.0, -0.5257311, 0.8506508, 0.0, -0.309017, 0.809017, -0.5, 0.0, 0.5257311,
    0.8506508, -0.309017, 0.809017, 0.5, 0.309017, 0.809017, 0.5, 0.5, 0.309017,
    0.809017, 0.5, -0.309017, 0.809017, 0.0, 0.0, 1.0, -0.5, 0.309017, 0.809017,
    -0.809017, 0.5, 0.309017, -0.809017, 0.5, -0.309017], dtype=np.float32).reshape(3, 21)

N_CORES = 8
RAYS_PER_CORE = 256
NS = 64           # samples per ray
NL = 16           # frequency levels
NF = 26           # padded features: 0..20 bg, 21 pad, 22..24 fg, 25 pad
NQ = 16           # samples per quarter
NO = 8            # samples per octant
OBW = 832         # padded out width per sample: 64 fgs + 64 fgc + 352 bgs + 352 bgc
FOUT = 768


# ---------------------------------------------------------------------------
# walrus workarounds (same as v1)
# ---------------------------------------------------------------------------

_PATCHED = False


def _apply_patches():
    """1) split >1 sem-waits per instruction (this walrus rejects multi-wait
    instructions);  2) rewrite sentinel Arctan activations into Sin2pi."""
    global _PATCHED
    if _PATCHED:
        return
    _PATCHED = True

    import concourse.bass2jax as bass2jax

    orig_compile = bass2jax.compile_bir_kernel

    def patched_compile(bir_json, tmpdir, neff_name="file.neff"):
        if isinstance(bir_json, bytes):
            bir_json = bir_json.replace(b'"func":"Arctan"', b'"func":"Sin2pi"')
        else:
            bir_json = bir_json.replace('"func":"Arctan"', '"func":"Sin2pi"')
        return orig_compile(bir_json, tmpdir, neff_name=neff_name)

    bass2jax.compile_bir_kernel = patched_compile


_waitsplit_ctr = [0]


def _split_sync_waits(nc, max_waits=1):
    n_split = 0
    for fn in nc.m.functions:
        for bb in fn.blocks:
            il = bb.instructions
            i = 0
            while i < len(il):
                ins = il[i]
                si = ins.sync_info
                waits = list(si.on_wait) if si is not None else []
                if len(waits) > max_waits:
                    extra, keep = waits[:-max_waits], waits[-max_waits:]
                    pos = i
                    for j in range(0, len(extra), max_waits):
                        chunk = extra[j:j + max_waits]
                        _waitsplit_ctr[0] += 1
                        nop = mybir.InstNoOp(
                            name=f"waitsplit_{_waitsplit_ctr[0]}", ins=[], outs=[])
                        nop.engine = ins.engine
                        nop.sync_info = mybir.SyncInfo(on_wait=chunk, on_update=[])
                        nc.register_instruction(nop, overwrite=True)
                        il.insert(pos, nop)
                        pos += 1
                        i += 1
                    ins.sync_info = mybir.SyncInfo(
                        on_wait=keep, on_update=list(si.on_update))
                    n_split += 1
                i += 1
    return n_split


# ---------------------------------------------------------------------------
# AP helpers
# ---------------------------------------------------------------------------

def _ap(base, offset_elems, dims):
    """Custom AP over a tile/AP: keep partition dim, replace free dims."""
    return bass.AP(tensor=base.tensor, offset=base.offset + offset_elems,
                   ap=[base.ap[0]] + [list(d) for d in dims])


# ---------------------------------------------------------------------------
# kernel body
# ---------------------------------------------------------------------------

def _moments2(nc, wide, zc, r2, out_m):
    """Frustum moments for fg AND bg in one 128-wide pass.
    zc: [128, 130] = [fg_z (65) | bg_z (65)].  out_m: [128, 3*128] laid out
    [t_mean2 fg|bg][t_var fg|bg][r_var fg|bg].  r2 = radii^2 [128, 1]."""
    N2 = 2 * NS
    t0 = _ap(zc[:], 0, [[NS + 1, 2], [1, NS]])
    t1 = _ap(zc[:], 1, [[NS + 1, 2], [1, NS]])
    sm = wide.tile([128, N2], F32, tag="mo_a")
    nc.vector.tensor_tensor(out=sm[:], in0=t0, in1=t1, op=OP.add)
    df = wide.tile([128, N2], F32, tag="mo_b")
    nc.vector.tensor_tensor(out=df[:], in0=t1, in1=t0, op=OP.subtract)
    sm2 = wide.tile([128, N2], F32, tag="mo_c")
    nc.vector.tensor_tensor(out=sm2[:], in0=sm[:], in1=sm[:], op=OP.mult)
    df2 = wide.tile([128, N2], F32, tag="mo_d")
    nc.vector.tensor_tensor(out=df2[:], in0=df[:], in1=df[:], op=OP.mult)
    # denom4 = 3*sm2 + df2
    den4 = wide.tile([128, N2], F32, tag="mo_e")
    nc.vector.scalar_tensor_tensor(out=den4[:], in0=sm2[:], scalar=3.0,
                                   in1=df2[:], op0=OP.mult, op1=OP.add)
    rden4 = wide.tile([128, N2], F32, tag="mo_f")
    nc.vector.reciprocal(out=rden4[:], in_=den4[:])
    u1 = wide.tile([128, N2], F32, tag="mo_g")
    nc.vector.tensor_tensor(out=u1[:], in0=df2[:], in1=rden4[:], op=OP.mult)
    # t_mean2 = sm * (1 + 2*u1)
    tmp = wide.tile([128, N2], F32, tag="mo_h")
    nc.vector.tensor_scalar(out=tmp[:], in0=u1[:], scalar1=2.0, scalar2=1.0,
                            op0=OP.mult, op1=OP.add)
    nc.vector.tensor_tensor(out=out_m[:, 0:N2], in0=sm[:], in1=tmp[:], op=OP.mult)
    # t_var = df2/12 - (4/15) * u1^2 * (den4 - 1.25*df2)
    u1sq = wide.tile([128, N2], F32, tag="mo_h")
    nc.vector.tensor_tensor(out=u1sq[:], in0=u1[:], in1=u1[:], op=OP.mult)
    g2 = wide.tile([128, N2], F32, tag="mo_a")
    nc.vector.scalar_tensor_tensor(out=g2[:], in0=df2[:], scalar=-1.25,
                                   in1=den4[:], op0=OP.mult, op1=OP.add)
    g3 = wide.tile([128, N2], F32, tag="mo_c")
    nc.vector.tensor_tensor(out=g3[:], in0=u1sq[:], in1=g2[:], op=OP.mult)
    g5 = wide.tile([128, N2], F32, tag="mo_e")
    nc.vector.tensor_scalar_mul(out=g5[:], in0=df2[:], scalar1=float(1.0 / 12.0))
    nc.vector.scalar_tensor_tensor(out=out_m[:, N2:2 * N2], in0=g3[:],
                                   scalar=float(-4.0 / 15.0),
                                   in1=g5[:], op0=OP.mult, op1=OP.add)
    # r_var = r2 * (sm2/16 + (5/48)*df2 - (1/15)*u1*df2)
    h1 = wide.tile([128, N2], F32, tag="mo_a")
    nc.vector.tensor_tensor(out=h1[:], in0=u1[:], in1=df2[:], op=OP.mult)
    h2 = wide.tile([128, N2], F32, tag="mo_c")
    nc.vector.tensor_scalar_mul(out=h2[:], in0=sm2[:], scalar1=float(1.0 / 16.0))
    h4 = wide.tile([128, N2], F32, tag="mo_e")
    nc.vector.scalar_tensor_tensor(out=h4[:], in0=df2[:], scalar=float(5.0 / 48.0),
                                   in1=h2[:], op0=OP.mult, op1=OP.add)
    h5 = wide.tile([128, N2], F32, tag="mo_a")
    nc.vector.scalar_tensor_tensor(out=h5[:], in0=h1[:], scalar=float(-1.0 / 15.0),
                                   in1=h4[:], op0=OP.mult, op1=OP.add)
    nc.vector.tensor_scalar_mul(out=out_m[:, 2 * N2:3 * N2], in0=h5[:], scalar1=r2[:])


def build_kernel():
    """Build the 8-core SPMD Bass module (per-core: 256 rays)."""
    _apply_patches()
    nc = bass.Bass(dynamic_dma_scratch_size=4096)

    ray_o = nc.dram_tensor("ray_o", [RAYS_PER_CORE, 3], F32, kind="ExternalInput")
    ray_d = nc.dram_tensor("ray_d", [RAYS_PER_CORE, 3], F32, kind="ExternalInput")
    fg_z = nc.dram_tensor("fg_z", [RAYS_PER_CORE, NS + 1], F32, kind="ExternalInput")
    bg_z = nc.dram_tensor("bg_z", [RAYS_PER_CORE, NS + 1], F32, kind="ExternalInput")
    radii = nc.dram_tensor("radii", [RAYS_PER_CORE, 1], F32, kind="ExternalInput")
    pconst = nc.dram_tensor("pconst", [1, 84], F32, kind="ExternalInput")
    out = nc.dram_tensor("out", [RAYS_PER_CORE, NS * OBW], BF16, kind="ExternalOutput")

    with tile.TileContext(nc) as tc:
        import contextlib
        ctx = contextlib.ExitStack()
        with ctx:
            consts = ctx.enter_context(tc.tile_pool(name="consts", bufs=1))
            base = ctx.enter_context(tc.tile_pool(name="base", bufs=1))
            wide = ctx.enter_context(tc.tile_pool(name="wide", bufs=2))
            upool = ctx.enter_context(tc.tile_pool(name="upool", bufs=1))
            uhpool = ctx.enter_context(tc.tile_pool(name="uhpool", bufs=2))
            spool = ctx.enter_context(tc.tile_pool(name="spool", bufs=2))
            cpool = ctx.enter_context(tc.tile_pool(name="cpool", bufs=2))
            epool = ctx.enter_context(tc.tile_pool(name="epool", bufs=2))
            obpool = ctx.enter_context(tc.tile_pool(name="obpool", bufs=2))

            # constants
            pc = consts.tile([128, 84], F32)
            pca = pconst[:, :]
            nc.sync.dma_start(out=pc[:], in_=bass.AP(
                tensor=pca.tensor, offset=pca.offset, ap=[[0, 128], [1, 84]]))
            magic_u = consts.tile([128, 1], U32)
            nc.vector.memset(magic_u, RSQRT_MAGIC)
            quarter = consts.tile([128, 1], F32)
            nc.vector.memset(quarter, 0.25)

            for t in range(2):
                r0 = t * 128

                # ---------------- load inputs ----------------
                zc = base.tile([128, 2 * (NS + 1)], F32, tag="zc")
                nc.sync.dma_start(out=zc[:, 0:NS + 1], in_=fg_z[r0:r0 + 128, :])
                nc.sync.dma_start(out=zc[:, NS + 1:2 * (NS + 1)], in_=bg_z[r0:r0 + 128, :])
                o3 = base.tile([128, 3], F32, tag="o3")
                nc.sync.dma_start(out=o3[:], in_=ray_o[r0:r0 + 128, :])
                d3 = base.tile([128, 3], F32, tag="d3")
                nc.sync.dma_start(out=d3[:], in_=ray_d[r0:r0 + 128, :])
                rad = base.tile([128, 1], F32, tag="rad")
                nc.sync.dma_start(out=rad[:], in_=radii[r0:r0 + 128, :])

                # ---------------- per-ray scalars ----------------
                r2 = base.tile([128, 1], F32, tag="r2")
                nc.vector.tensor_tensor(out=r2[:], in0=rad[:], in1=rad[:], op=OP.mult)
                dk2 = base.tile([128, 3], F32, tag="dk2")
                nc.vector.tensor_tensor(out=dk2[:], in0=d3[:], in1=d3[:], op=OP.mult)
                dmag = base.tile([128, 1], F32, tag="dmag")
                nc.vector.tensor_tensor(out=dmag[:], in0=dk2[:, 0:1], in1=dk2[:, 1:2], op=OP.add)
                nc.vector.tensor_tensor(out=dmag[:], in0=dmag[:], in1=dk2[:, 2:3], op=OP.add)
                nc.vector.tensor_scalar_max(out=dmag[:], in0=dmag[:], scalar1=1e-8)
                rdmag = base.tile([128, 1], F32, tag="rdmag")
                nc.vector.reciprocal(out=rdmag[:], in_=dmag[:])
                hd3 = base.tile([128, 3], F32, tag="hd3")
                nc.vector.tensor_scalar_mul(out=hd3[:], in0=d3[:], scalar1=0.5)
                # inv2pi-folded copies for the fg angle path
                hd3i = base.tile([128, 3], F32, tag="hd3i")
                nc.vector.tensor_scalar_mul(out=hd3i[:], in0=hd3[:], scalar1=INV2PI)
                o3i = base.tile([128, 3], F32, tag="o3i")
                nc.vector.tensor_scalar_mul(out=o3i[:], in0=o3[:], scalar1=INV2PI)

                # e = d @ P  [128, 21], esq
                e21 = base.tile([128, 21], F32, tag="e21")
                nc.vector.tensor_scalar_mul(out=e21[:], in0=pc[:, 0:21], scalar1=d3[:, 0:1])
                tmp21 = base.tile([128, 21], F32, tag="tmp21")
                nc.vector.tensor_scalar_mul(out=tmp21[:], in0=pc[:, 21:42], scalar1=d3[:, 1:2])
                nc.vector.tensor_tensor(out=e21[:], in0=e21[:], in1=tmp21[:], op=OP.add)
                nc.vector.tensor_scalar_mul(out=tmp21[:], in0=pc[:, 42:63], scalar1=d3[:, 2:3])
                nc.vector.tensor_tensor(out=e21[:], in0=e21[:], in1=tmp21[:], op=OP.add)
                esq = base.tile([128, 21], F32, tag="esq")
                nc.vector.tensor_tensor(out=esq[:], in0=e21[:], in1=e21[:], op=OP.mult)

                # ---------------- moments (fg+bg in one 128-wide pass) -------
                mom = base.tile([128, 6 * NS], F32, tag="mom")
                _moments2(nc, wide, zc, r2, mom)
                tm2f = mom[:, 0:NS]
                tm2b = mom[:, NS:2 * NS]
                tvf = mom[:, 2 * NS:3 * NS]
                tvb = mom[:, 3 * NS:4 * NS]
                rvf = mom[:, 4 * NS:5 * NS]
                rvb = mom[:, 5 * NS:6 * NS]

                # g0 (angle/2pi) and yv0 (variance) tiles, [s(64)][f(26)]
                g0 = base.tile([128, NS * NF], F32, tag="g0")
                nc.gpsimd.memset(g0, 0.0)
                yv0 = base.tile([128, NS * NF], F32, tag="yv0")
                nc.gpsimd.memset(yv0, 0.0)

                # ---------------- fg: mean + cov_diag into cols 22..24 -------
                alf = wide.tile([128, NS], F32, tag="mo_b")
                nc.vector.tensor_scalar_mul(out=alf[:], in0=rvf, scalar1=rdmag[:])
                nc.vector.tensor_tensor(out=alf[:], in0=tvf, in1=alf[:], op=OP.subtract)
                for k in range(3):
                    # g0_fg = (tm2f * hd3_k + o_k) * inv2pi (folded constants)
                    nc.vector.tensor_scalar(
                        out=_ap(g0[:], 22 + k, [[NF, NS]]), in0=tm2f,
                        scalar1=hd3i[:, k:k + 1], scalar2=o3i[:, k:k + 1],
                        op0=OP.mult, op1=OP.add)
                    # cd_k = alf * dk2_k + rvf
                    nc.vector.scalar_tensor_tensor(
                        out=_ap(yv0[:], 22 + k, [[NF, NS]]), in0=alf[:],
                        scalar=dk2[:, k:k + 1], in1=rvf, op0=OP.mult, op1=OP.add)

                # ---------------- bg: contraction scalars ----------------
                X = base.tile([128, 3 * NS], F32, tag="X")          # [k*64+s]
                for k in range(3):
                    nc.vector.tensor_scalar(
                        out=X[:, k * NS:(k + 1) * NS], in0=tm2b,
                        scalar1=hd3[:, k:k + 1], scalar2=o3[:, k:k + 1],
                        op0=OP.mult, op1=OP.add)
                s2 = base.tile([128, NS], F32, tag="s2")
                nc.vector.tensor_tensor(out=s2[:], in0=X[:, 0:NS], in1=X[:, 0:NS], op=OP.mult)
                w0 = wide.tile([128, NS], F32, tag="mo_a")
                nc.vector.tensor_tensor(out=w0[:], in0=X[:, NS:2 * NS], in1=X[:, NS:2 * NS], op=OP.mult)
                nc.vector.tensor_tensor(out=s2[:], in0=s2[:], in1=w0[:], op=OP.add)
                nc.vector.tensor_tensor(out=w0[:], in0=X[:, 2 * NS:3 * NS], in1=X[:, 2 * NS:3 * NS], op=OP.mult)
                nc.vector.tensor_tensor(out=s2[:], in0=s2[:], in1=w0[:], op=OP.add)
                # h = d . X
                h = base.tile([128, NS], F32, tag="h")
                nc.vector.tensor_scalar_mul(out=h[:], in0=X[:, 0:NS], scalar1=d3[:, 0:1])
                nc.vector.scalar_tensor_tensor(out=h[:], in0=X[:, NS:2 * NS],
                                               scalar=d3[:, 1:2], in1=h[:],
                                               op0=OP.mult, op1=OP.add)
                nc.vector.scalar_tensor_tensor(out=h[:], in0=X[:, 2 * NS:3 * NS],
                                               scalar=d3[:, 2:3], in1=h[:],
                                               op0=OP.mult, op1=OP.add)

                # rsqrt(s2): magic seed + 4 Newton iterations
                rn0 = base.tile([128, NS], F32, tag="rn0")
                seed_u = wide.tile([128, NS], U32, tag="mo_a")
                nc.vector.tensor_scalar(out=seed_u[:], in0=s2[:].bitcast(U32),
                                        scalar1=1, scalar2=None,
                                        op0=OP.logical_shift_right)
                nc.vector.tensor_tensor(
                    out=rn0[:].bitcast(U32),
                    in0=_ap(magic_u[:], 0, [[0, NS]]),
                    in1=seed_u[:], op=OP.subtract)
                for _ in range(3):
                    nr = wide.tile([128, NS], F32, tag="mo_b")
                    nc.vector.tensor_tensor(out=nr[:], in0=s2[:], in1=rn0[:], op=OP.mult)
                    nc.vector.tensor_tensor(out=nr[:], in0=nr[:], in1=rn0[:], op=OP.mult)
                    nc.vector.tensor_scalar(out=nr[:], in0=nr[:], scalar1=-0.5,
                                            scalar2=1.5, op0=OP.mult, op1=OP.add)
                    nc.vector.tensor_tensor(out=rn0[:], in0=rn0[:], in1=nr[:], op=OP.mult)

                n0 = base.tile([128, NS], F32, tag="n0")
                nc.vector.tensor_tensor(out=n0[:], in0=s2[:], in1=rn0[:], op=OP.mult)
                rn = base.tile([128, NS], F32, tag="rn")
                nc.vector.tensor_scalar(out=rn[:], in0=rn0[:], scalar1=-TINY,
                                        scalar2=1.0, op0=OP.mult, op1=OP.add)
                nc.vector.tensor_tensor(out=rn[:], in0=rn0[:], in1=rn[:], op=OP.mult)
                a_ = base.tile([128, NS], F32, tag="a")
                nc.vector.tensor_scalar(out=a_[:], in0=rn[:], scalar1=-1.0,
                                        scalar2=2.0, op0=OP.mult, op1=OP.add)
                nc.vector.tensor_tensor(out=a_[:], in0=rn[:], in1=a_[:], op=OP.mult)
                b_ = base.tile([128, NS], F32, tag="b")
                nc.vector.tensor_scalar_add(out=b_[:], in0=rn[:], scalar1=-1.0)
                t2_ = wide.tile([128, NS], F32, tag="mo_a")
                nc.vector.tensor_tensor(out=t2_[:], in0=rn[:], in1=rn0[:], op=OP.mult)
                nc.vector.tensor_tensor(out=t2_[:], in0=t2_[:], in1=rn[:], op=OP.mult)
                nc.vector.tensor_tensor(out=b_[:], in0=t2_[:], in1=b_[:], op=OP.mult)
                nc.vector.tensor_scalar_mul(out=b_[:], in0=b_[:], scalar1=2.0)

                # alpha_b, A coefficients
                alb = base.tile([128, NS], F32, tag="alb")
                nc.vector.tensor_scalar_mul(out=alb[:], in0=rvb, scalar1=rdmag[:])
                nc.vector.tensor_tensor(out=alb[:], in0=tvb, in1=alb[:], op=OP.subtract)
                bh = base.tile([128, NS], F32, tag="bh")
                nc.vector.tensor_tensor(out=bh[:], in0=b_[:], in1=h[:], op=OP.mult)
                asq = wide.tile([128, NS], F32, tag="mo_a")
                nc.vector.tensor_tensor(out=asq[:], in0=a_[:], in1=a_[:], op=OP.mult)
                A1 = base.tile([128, NS], F32, tag="A1")
                nc.vector.tensor_tensor(out=A1[:], in0=alb[:], in1=asq[:], op=OP.mult)
                A4 = base.tile([128, NS], F32, tag="A4")
                nc.vector.tensor_tensor(out=A4[:], in0=rvb, in1=asq[:], op=OP.mult)
                A2 = base.tile([128, NS], F32, tag="A2")
                nc.vector.tensor_tensor(out=A2[:], in0=alb[:], in1=a_[:], op=OP.mult)
                nc.vector.tensor_tensor(out=A2[:], in0=A2[:], in1=bh[:], op=OP.mult)
                nc.vector.tensor_scalar_mul(out=A2[:], in0=A2[:], scalar1=2.0)
                # A3 = alb*bh^2 + rvb*(2ab + (b*n0)^2)
                A3 = base.tile([128, NS], F32, tag="A3")
                bn = wide.tile([128, NS], F32, tag="mo_b")
                nc.vector.tensor_tensor(out=bn[:], in0=b_[:], in1=n0[:], op=OP.mult)
                nc.vector.tensor_tensor(out=bn[:], in0=bn[:], in1=bn[:], op=OP.mult)
                ab = wide.tile([128, NS], F32, tag="mo_c")
                nc.vector.tensor_tensor(out=ab[:], in0=a_[:], in1=b_[:], op=OP.mult)
                nc.vector.scalar_tensor_tensor(out=bn[:], in0=ab[:], scalar=2.0,
                                               in1=bn[:], op0=OP.mult, op1=OP.add)
                nc.vector.tensor_tensor(out=A3[:], in0=rvb, in1=bn[:], op=OP.mult)
                bh2 = wide.tile([128, NS], F32, tag="mo_a")
                nc.vector.tensor_tensor(out=bh2[:], in0=bh[:], in1=bh[:], op=OP.mult)
                nc.vector.tensor_tensor(out=bh2[:], in0=alb[:], in1=bh2[:], op=OP.mult)
                nc.vector.tensor_tensor(out=A3[:], in0=A3[:], in1=bh2[:], op=OP.add)
                # ai = a * inv2pi (fold the angle scale into a)
                ai = base.tile([128, NS], F32, tag="ai")
                nc.vector.tensor_scalar_mul(out=ai[:], in0=a_[:], scalar1=INV2PI)

                # ---------------- c = X . p_q   [s*21+q] (s-major) ----------
                c = base.tile([128, 21 * NS], F32, tag="c")
                w1 = base.tile([128, NS * NF], F32, tag="w1")  # scratch (1664)
                w2 = base.tile([128, 21 * NS], F32, tag="w2")
                GP0 = _ap(pc[:], 0, [[0, NS], [1, 21]])
                GP1 = _ap(pc[:], 21, [[0, NS], [1, 21]])
                GP2 = _ap(pc[:], 42, [[0, NS], [1, 21]])
                X0 = _ap(X[:], 0, [[1, NS], [0, 21]])
                X1 = _ap(X[:], NS, [[1, NS], [0, 21]])
                X2 = _ap(X[:], 2 * NS, [[1, NS], [0, 21]])
                w1c = _ap(w1[:], 0, [[1, 21 * NS]])
                nc.gpsimd.tensor_tensor(out=c[:], in0=X0, in1=GP0, op=OP.mult)
                nc.gpsimd.tensor_tensor(out=w1c, in0=X1, in1=GP1, op=OP.mult)
                nc.gpsimd.tensor_tensor(out=c[:], in0=c[:], in1=w1c, op=OP.add)
                nc.gpsimd.tensor_tensor(out=w1c, in0=X2, in1=GP2, op=OP.mult)
                nc.gpsimd.tensor_tensor(out=c[:], in0=c[:], in1=w1c, op=OP.add)

                # ---------------- yv0 / g0 bg columns ----------------
                # yv0_q = (A2*e + A3*c)*c + (A1*esq + A4*w);  g0_q = ai*c
                A2b = _ap(A2[:], 0, [[1, NS], [0, 21]])
                A3b = _ap(A3[:], 0, [[1, NS], [0, 21]])
                A1b = _ap(A1[:], 0, [[1, NS], [0, 21]])
                A4b = _ap(A4[:], 0, [[1, NS], [0, 21]])
                aib = _ap(ai[:], 0, [[1, NS], [0, 21]])
                e_b = _ap(e21[:], 0, [[0, NS], [1, 21]])
                esq_b = _ap(esq[:], 0, [[0, NS], [1, 21]])
                w_b = _ap(pc[:], 63, [[0, NS], [1, 21]])
                w2c = _ap(w2[:], 0, [[1, 21 * NS]])
                nc.gpsimd.tensor_tensor(out=w1c, in0=A2b, in1=e_b, op=OP.mult)
                nc.gpsimd.tensor_tensor(out=w2c, in0=A3b, in1=c[:], op=OP.mult)
                nc.gpsimd.tensor_tensor(out=w1c, in0=w1c, in1=w2c, op=OP.add)
                nc.gpsimd.tensor_tensor(out=w1c, in0=w1c, in1=c[:], op=OP.mult)
                nc.gpsimd.tensor_tensor(out=w2c, in0=A1b, in1=esq_b, op=OP.mult)
                nc.gpsimd.tensor_tensor(out=w2c, in0=w2c, in1=w1c, op=OP.add)
                yv_bg = _ap(yv0[:], 0, [[NF, NS], [1, 21]])
                nc.gpsimd.tensor_tensor(out=w1c, in0=A4b, in1=w_b, op=OP.mult)
                nc.gpsimd.tensor_tensor(out=yv_bg, in0=w1c, in1=w2c, op=OP.add)
                # g0 bg = ai * c  (strided out into [s][f] cols 0..20)
                g0_bg = _ap(g0[:], 0, [[NF, NS], [1, 21]])
                nc.gpsimd.tensor_tensor(out=g0_bg, in0=aib, in1=c[:], op=OP.mult)

                # ---------------- frac + int angle base ----------------
                # q = round(g0); f0 = g0 - q; u0 = f0 * 2^32 (int32)
                qr = base.tile([128, NS * NF], F32, tag="w1")
                nc.vector.tensor_scalar(out=qr[:], in0=g0[:], scalar1=MAGIC_RND,
                                        scalar2=MAGIC_RND, op0=OP.add, op1=OP.subtract)
                f0 = base.tile([128, NS * NF], F32, tag="f0")
                nc.gpsimd.tensor_tensor(out=f0[:], in0=g0[:], in1=qr[:], op=OP.subtract)

                # ---------------- per-quarter angle cascade + per-octant -----
                for qq in range(4):
                    s_q = qq * NQ
                    if qq % 2 == 0:
                        # exp levels for this half: E[s(32)][j][f] bf16
                        E = epool.tile([128, 32 * NL * NF], BF16, tag="E")
                        for j in range(NL):
                            nc.scalar.activation(
                                out=_ap(E[:], j * NF, [[NL * NF, 32], [1, NF]]),
                                in_=_ap(yv0[:], s_q * NF, [[NF, 32], [1, NF]]),
                                func=AF.Exp, scale=float(-0.5 * (4.0 ** j)))
                        e_base = s_q
                    uq = upool.tile([128, NQ * NL * NF], I32, tag="uq")
                    # u0 into level-0 slots
                    nc.vector.tensor_scalar_mul(
                        out=_ap(uq[:], 0, [[NL * NF, NQ], [1, NF]]),
                        in0=f0[:, s_q * NF:(s_q + NQ) * NF],
                        scalar1=float(2.0 ** 32))
                    # binary-doubling cascade: [1]=[0]<<1; [2:4]=[0:2]<<2;
                    # [4:8]=[0:4]<<4; [8:16]=[0:8]<<8
                    for (src, w, sh) in ((0, 1, 1), (0, 2, 2), (0, 4, 4), (0, 8, 8)):
                        nc.vector.tensor_scalar(
                            out=_ap(uq[:], w * NF, [[NL * NF, NQ], [1, w * NF]]),
                            in0=_ap(uq[:], src * NF, [[NL * NF, NQ], [1, w * NF]]),
                            scalar1=sh, scalar2=None, op0=OP.logical_shift_left)

                    for oo in range(2):
                        s_o = s_q + oo * NO         # absolute first sample
                        uo = uq[:, oo * NO * NL * NF:(oo + 1) * NO * NL * NF]
                        # sin: ACT reads int32 directly
                        S = spool.tile([128, NO * NL * NF], BF16, tag="S")
                        nc.scalar.activation(out=S[:], in_=uo, func=AF.Arctan,
                                             scale=float(2.0 ** -32))
                        # cos: fused cast to fp16 angle, 4x abs, ACT
                        uh = uhpool.tile([128, NO * NL * NF], F16, tag="uh")
                        nc.vector.tensor_scalar_mul(out=uh[:], in0=uo,
                                                    scalar1=float(2.0 ** -32))
                        nc.vector.tensor_scalar(out=uh[:].bitcast(U16), in0=uh[:].bitcast(U16),
                                                scalar1=0x7FFF, scalar2=None,
                                                op0=OP.bitwise_and)
                        C = cpool.tile([128, NO * NL * NF], BF16, tag="C")
                        nc.scalar.activation(out=C[:], in_=uh[:], func=AF.Arctan,
                                             scale=-1.0, bias=quarter[:])

                        # products into padded out block [s][832]
                        ob = obpool.tile([128, NO * OBW], BF16, tag="ob")
                        eoff = (s_o - e_base) * NL * NF
                        # bg sin: f 0..21 (run 22)
                        nc.vector.tensor_tensor(
                            out=_ap(ob[:], 128, [[OBW, NO], [22, NL], [1, 22]]),
                            in0=_ap(S[:], 0, [[NL * NF, NO], [NF, NL], [1, 22]]),
                            in1=_ap(E[:], eoff, [[NL * NF, NO], [NF, NL], [1, 22]]),
                            op=OP.mult)
                        # bg cos
                        nc.vector.tensor_tensor(
                            out=_ap(ob[:], 480, [[OBW, NO], [22, NL], [1, 22]]),
                            in0=_ap(C[:], 0, [[NL * NF, NO], [NF, NL], [1, 22]]),
                            in1=_ap(E[:], eoff, [[NL * NF, NO], [NF, NL], [1, 22]]),
                            op=OP.mult)
                        # fg sin: f 22..25 (run 4)
                        nc.vector.tensor_tensor(
                            out=_ap(ob[:], 0, [[OBW, NO], [4, NL], [1, 4]]),
                            in0=_ap(S[:], 22, [[NL * NF, NO], [NF, NL], [1, 4]]),
                            in1=_ap(E[:], eoff + 22, [[NL * NF, NO], [NF, NL], [1, 4]]),
                            op=OP.mult)
                        # fg cos
                        nc.vector.tensor_tensor(
                            out=_ap(ob[:], 64, [[OBW, NO], [4, NL], [1, 4]]),
                            in0=_ap(C[:], 22, [[NL * NF, NO], [NF, NL], [1, 4]]),
                            in1=_ap(E[:], eoff + 22, [[NL * NF, NO], [NF, NL], [1, 4]]),
                            op=OP.mult)

                        # DMA out
                        oa = out[:, :]
                        nc.sync.dma_start(
                            out=bass.AP(tensor=oa.tensor,
                                        offset=oa.offset + r0 * NS * OBW + s_o * OBW,
                                        ap=[[NS * OBW, 128], [1, NO * OBW]]),
                            in_=ob[:])

    _split_sync_waits(nc)
    return nc


# ---------------------------------------------------------------------------
# entry point
# ---------------------------------------------------------------------------

_NC_CACHE = []


def kernel(ray_o, ray_d, fg_z_vals, bg_z_vals, radii):
    from concourse.bass_utils import run_bass_kernel_spmd

    if not _NC_CACHE:
        _NC_CACHE.append(build_kernel())
    nc = _NC_CACHE[0]

    pconst = np.concatenate(
        [P_BASIS.reshape(-1), (P_BASIS * P_BASIS).sum(axis=0)]).astype(np.float32)[None, :]

    in_maps = []
    for cidx in range(N_CORES):
        sl = slice(cidx * RAYS_PER_CORE, (cidx + 1) * RAYS_PER_CORE)
        in_maps.append({
            "ray_o": np.ascontiguousarray(ray_o[sl]).astype(np.float32, copy=False),
            "ray_d": np.ascontiguousarray(ray_d[sl]).astype(np.float32, copy=False),
            "fg_z": np.ascontiguousarray(fg_z_vals[sl]).astype(np.float32, copy=False),
            "bg_z": np.ascontiguousarray(bg_z_vals[sl]).astype(np.float32, copy=False),
            "radii": np.ascontiguousarray(radii[sl]).astype(np.float32, copy=False),
            "pconst": pconst,
        })

    res = run_bass_kernel_spmd(nc, in_maps, core_ids=list(range(N_CORES)))
    full = np.concatenate(
        [np.asarray(res.results[i]["out"]) for i in range(N_CORES)], axis=0)
    v = full.reshape(2048, NS, OBW).astype(np.float32)
    fs = v[:, :, 0:64].reshape(2048, NS, 16, 4)[:, :, :, :3].reshape(2048, NS, 48)
    fc = v[:, :, 64:128].reshape(2048, NS, 16, 4)[:, :, :, :3].reshape(2048, NS, 48)
    bs = v[:, :, 128:480].reshape(2048, NS, 16, 22)[:, :, :, :21].reshape(2048, NS, 336)
    bc = v[:, :, 480:832].reshape(2048, NS, 16, 22)[:, :, :, :21].reshape(2048, NS, 336)
    return np.ascontiguousarray(
        np.concatenate([fs, fc, bs, bc], axis=-1), dtype=np.float32)
